# revision 25
# baseline (speedup 1.0000x reference)
"""Causal self-attention with rotary embeddings on 8 Trainium2 NeuronCores.

v2: 2-way batch data-parallel x 4-way head tensor-parallel.
Core m handles batch m//4 and heads 4*(m%4)..4*(m%4)+3 (two pairs).
Each core computes qkv for its 4 heads, rotary, causal attention, and a
partial output projection (its 256 rows of w_proj) over its batch's
2048 tokens; the host sums 4 partials per batch.

Device-side layout (per core; heads within a pair at partitions 0-63 / 64-127):
  - Everything "transposed": Q^T/K^T stored [d(128), pair, t(2048)].
  - Scores S^T = K_blk @ Q^T -> [k(128), q]; the two heads of a pair issue
    back-to-back K=64 matmuls into different PE row groups (tile_position
    (0,0)/(64,0)) so they run concurrently.
  - exp on the scalar engine (PSUM fp32 -> SBUF fp16 pt tiles); softmax's
    k-sum folded into P@V via a ones-augmented V column (denominator row).
  - Rotary via pair-swap permutation matmul: rot(q) = cos*q + sin_sgn*(Pswap@q).
  - V transposed to t-major via hardware DMA transpose (XBAR), not the PE.
  - PV accumulates the causal triangle with narrow diagonal-block matmuls
    (has_written semantics preserve untouched columns); no zero-padding.
  - Emission interleaves pair-1 QK, V-groups and PV/projection chunks into
    the exp-paced score stream so the PE never head-of-line blocks.
"""

import numpy as np

B, T, C, H = 2, 2048, 1024, 16
HD = C // H            # 64
N_CORES = 8
DP = 2                 # batch shards
TPC = 4                # head-group shards
NP = 2                 # pairs per core (4 heads)
TC = 512               # t-chunk
NTC = T // TC          # 4
KB = 128               # k-block
NKB = T // KB          # 16
QC = 512               # q-chunk for PV
NQC = T // QC          # 4

_CACHE = {}


def _build_bass():
    import concourse.bacc as bacc
    import concourse.mybir as mybir
    import concourse.tile as tile
    from concourse.masks import make_identity

    f16 = mybir.dt.float16
    f32 = mybir.dt.float32

    nc = bacc.Bacc()

    xT = nc.dram_tensor("xT", [C, T], f16, kind="ExternalInput")
    wqkv = nc.dram_tensor("wqkv", [C, 6 * 128], f16, kind="ExternalInput")
    wp = nc.dram_tensor("wp", [2 * 128, C], f16, kind="ExternalInput")
    cos_d = nc.dram_tensor("cos_d", [128, T], f16, kind="ExternalInput")
    sin_d = nc.dram_tensor("sin_d", [128, T], f16, kind="ExternalInput")
    pswap = nc.dram_tensor("pswap", [128, 128], f16, kind="ExternalInput")
    maskneg = nc.dram_tensor("maskneg", [128, 128], f16, kind="ExternalInput")
    y = nc.dram_tensor("y", [T, C], f16, kind="ExternalOutput")

    CCH = C // 128  # 8 contraction chunks

    with tile.TileContext(nc) as tc:
        with (
            tc.tile_pool(name="const", bufs=1) as const,
            tc.tile_pool(name="persist", bufs=1) as persist,
            tc.tile_pool(name="ptp", bufs=1) as ptp,
            tc.tile_pool(name="stream", bufs=2) as stream,
            tc.tile_pool(name="psum", bufs=1, space="PSUM") as psum,
        ):
            # ---- constants (ordered so the first QKV group starts ASAP) ----
            xT_r = xT.rearrange("(cc p) t -> p cc t", p=128)
            x_all = persist.tile([128, CCH, NTC, TC], f16)
            nc.sync.dma_start(out=x_all[:, :, 0, :], in_=xT_r[:, :, 0:TC])
            wqkv_sb = const.tile([128, CCH, 6, 128], f16)
            wqkv_r = wqkv.rearrange("(cc p) (g j) -> p cc g j", p=128, j=128)
            nc.sync.dma_start(out=wqkv_sb[:, :, 0, :], in_=wqkv_r[:, :, 0, :])
            nc.sync.dma_start(out=wqkv_sb[:, :, 1, :], in_=wqkv_r[:, :, 1, :])
            pswap_sb = const.tile([128, 128], f16)
            nc.sync.dma_start(out=pswap_sb, in_=pswap[:, :])
            cos_sb = const.tile([128, T], f16)
            nc.sync.dma_start(out=cos_sb, in_=cos_d[:, :])
            sin_sb = const.tile([128, T], f16)
            nc.sync.dma_start(out=sin_sb, in_=sin_d[:, :])
            for i in range(1, NTC):
                nc.sync.dma_start(out=x_all[:, :, i, :],
                                  in_=xT_r[:, :, i * TC:(i + 1) * TC])
            nc.sync.dma_start(out=wqkv_sb[:, :, 2:6, :],
                              in_=wqkv_r[:, :, 2:6, :])
            wp_sb = const.tile([128, 2, C], f16)
            nc.sync.dma_start(out=wp_sb, in_=wp.rearrange("(pp p) c -> p pp c",
                                                          p=128))
            # maskneg[k, q] = -100 where q < k, else 0 (added to diag scores
            # pre-exp via an identity matmul, so masking costs PE only)
            mneg_sb = const.tile([128, 128], f16)
            nc.sync.dma_start(out=mneg_sb, in_=maskneg[:, :])

            # ---- persistent tensors ----
            QrotT = persist.tile([128, NP, T], f16)
            KrotT = persist.tile([128, NP, T], f16)
            # V t-major per (pair, k-block): [V_A(64) | ones | V_B(64) | ones]
            Vaug = persist.tile([128, NP, NKB, 130], f16)
            Yn = persist.tile([128, NP, T], f16)

            # ---------- emission helpers ----------
            def emit_qk(i, p, gk):
                """QKV group for (chunk i, pair p, Q:gk=0/K:gk=1) + rotary."""
                ts = slice(i * TC, (i + 1) * TC)
                g = 2 * p + gk
                dst = QrotT if gk == 0 else KrotT
                acc = psum.tile([128, TC], f32, tag="acc", bufs=4, name="acc")
                for cc in range(CCH):
                    nc.tensor.matmul(
                        acc, wqkv_sb[:, cc, g, :], x_all[:, cc, i, :],
                        start=(cc == 0), stop=(cc == CCH - 1))
                graw = stream.tile([128, TC], f16, tag="graw", bufs=3)
                nc.vector.tensor_copy(graw, acc)
                swp = psum.tile([128, TC], f32, tag="acc", bufs=4, name="swp")
                nc.tensor.matmul(swp, pswap_sb, graw, start=True, stop=True)
                t1 = stream.tile([128, TC], f16, tag="t1")
                nc.vector.tensor_mul(t1, graw, cos_sb[:, ts])
                t2 = stream.tile([128, TC], f16, tag="t2")
                nc.vector.tensor_mul(t2, swp, sin_sb[:, ts])
                nc.vector.tensor_add(dst[:, p, ts], t1, t2)

            def emit_v(i, p):
                """V group for (chunk i, pair p): matmul + DMA transpose."""
                g = 4 + p
                acc = psum.tile([128, TC], f32, tag="acc", bufs=4, name="vacc")
                for cc in range(CCH):
                    nc.tensor.matmul(
                        acc, wqkv_sb[:, cc, g, :], x_all[:, cc, i, :],
                        start=(cc == 0), stop=(cc == CCH - 1))
                vtmp = stream.tile([128, TC], f16, tag="vtmp")
                nc.vector.tensor_copy(vtmp, acc)
                vt4 = stream.tile([128, 4, 128], f16, tag="vt4")
                nc.sync.dma_start_transpose(out=vt4[:, :, :], in_=vtmp)
                # vt4[tlo, thi, d]; d 0-63 head A, 64-127 head B
                Vr = Vaug.rearrange("p pp J (h x) -> p pp J h x", x=65)
                nc.vector.tensor_copy(
                    Vr[:, p, 4 * i:4 * i + 4, 0, 0:64], vt4[:, :, 0:64])
                nc.vector.tensor_copy(
                    Vr[:, p, 4 * i:4 * i + 4, 1, 0:64], vt4[:, :, 64:128])

            pt = {}  # (pair-local head h2, j) -> tile

            def emit_score_window(p, j, w0):
                """One 1024-wide score+exp window for both heads of pair p."""
                L = T - j * KB
                k0 = j * KB
                if w0 == 0:
                    ptA = ptp.tile([128, L], f16, tag=f"pt0_{j}", bufs=1,
                                   name="ptA")
                    ptB = ptp.tile([128, L], f16, tag=f"pt1_{j}", bufs=1,
                                   name="ptB")
                    pt[(0, j)] = ptA
                    pt[(1, j)] = ptB
                ptA, ptB = pt[(0, j)], pt[(1, j)]
                nw = min(1024, L - w0)
                stA = psum.tile([128, 1024], f32, tag="stA", bufs=1,
                                name="stA")
                stB = psum.tile([128, 1024], f32, tag="stB", bufs=1,
                                name="stB")
                for s0 in range(0, nw, 512):
                    ns = min(512, nw - s0)
                    q0 = k0 + w0 + s0
                    diag = (w0 == 0 and s0 == 0)
                    nc.tensor.matmul(
                        stA[:, s0:s0 + ns], KrotT[0:64, p, k0:k0 + 128],
                        QrotT[0:64, p, q0:q0 + ns], start=True,
                        stop=not diag)
                    nc.tensor.matmul(
                        stB[:, s0:s0 + ns], KrotT[64:128, p, k0:k0 + 128],
                        QrotT[64:128, p, q0:q0 + ns], start=True,
                        stop=not diag)
                    if diag:
                        # add -100 above the causal diagonal before exp
                        nc.tensor.matmul(stA[:, 0:128], ident, mneg_sb,
                                         start=False, stop=True)
                        nc.tensor.matmul(stB[:, 0:128], ident, mneg_sb,
                                         start=False, stop=True)
                nc.scalar.activation(
                    ptA[:, w0:w0 + nw], stA[:, 0:nw],
                    mybir.ActivationFunctionType.Exp)
                nc.scalar.activation(
                    ptB[:, w0:w0 + nw], stB[:, 0:nw],
                    mybir.ActivationFunctionType.Exp)

            def emit_pv_head(p, c, h2, yps_out):
                """PV accumulation for one head of one q-chunk (<=16 MMs)."""
                for _ in range(4):
                    nc.tensor.ldweights(weights=warm)
                jmax = 4 * c + 3
                ypst = psum.tile([128, QC], f32, tag="acc", bufs=4,
                                 name="yps")
                yps_out[h2] = ypst
                for j in range(jmax + 1):
                    lhsT = Vaug[:, p, j, h2 * 65:(h2 + 1) * 65]
                    off = c * QC - j * KB
                    if off >= 0:
                        rhs = pt[(h2, j)][:, off:off + QC]
                        out = ypst[0:65, :]
                    else:
                        rhs = pt[(h2, j)][:, 0:QC + off]
                        out = ypst[0:65, -off:QC]
                    nc.tensor.matmul(out, lhsT, rhs,
                                     start=(j == 0), stop=(j == jmax))

            def emit_norm(p, c, h2, ypst):
                """normalize rows 0-63 by the ones-row (64) -> Yn.
                Broadcast of 1/den via a K=1 PE matmul so gpsimd's FIFO
                (which runs the causal masks) is never on this chain."""
                dsb = stream.tile([128, QC], f32, tag="dsb")
                nc.vector.tensor_copy(dsb[0:1, :], ypst[64:65, :])
                recip = stream.tile([128, QC], f32, tag="recip")
                nc.vector.reciprocal_approx_fast(
                    out=recip[0:1, :], in_=dsb[0:1, :])
                bc = stream.tile([128, QC], f32, tag="bc")
                nc.gpsimd.partition_broadcast(bc[0:64, :], recip[0:1, :])
                if h2 == 0:
                    nc.vector.tensor_tensor(
                        out=Yn[0:64, p, c * QC:(c + 1) * QC],
                        in0=ypst[0:64, :], in1=bc[0:64, :],
                        op=mybir.AluOpType.mult)
                else:
                    ytmp = stream.tile([128, QC], f16, tag="ytmp")
                    nc.vector.tensor_tensor(
                        out=ytmp[0:64, :], in0=ypst[0:64, :],
                        in1=bc[0:64, :], op=mybir.AluOpType.mult)
                    nc.sync.dma_start(
                        out=Yn[64:128, p, c * QC:(c + 1) * QC],
                        in_=ytmp[0:64, :])

            def emit_proj(tt2):
                """Projection for a 256-token block (4 MMs, 2 evacs)."""
                for tt in range(2 * tt2, 2 * tt2 + 2):
                    for half in range(2):
                        pout = psum.tile([128, 512], f32, tag="acc",
                                         bufs=4, name="pout")
                        for pp in range(2):
                            nc.tensor.matmul(
                                pout, Yn[:, pp, tt * 128:(tt + 1) * 128],
                                wp_sb[:, pp, half * 512:(half + 1) * 512],
                                start=(pp == 0), stop=(pp == 1))
                        yout = stream.tile([128, 512], f16, tag="yo")
                        nc.vector.tensor_copy(yout, pout)
                        nc.sync.dma_start(
                            out=y[tt * 128:(tt + 1) * 128,
                                  half * 512:(half + 1) * 512],
                            in_=yout)

            # ---------- phase 1 lead-in: pair-0 Q + first K ----------
            # dependency-free LDWEIGHTS keep the PE-activity monitor busy
            # during the initial DMA wait so the first matmuls run at full
            # clock (HAM un-throttles after ~3.4us of sustained activity)
            warm = const.tile([128, 128], f16)
            nc.gpsimd.memset(warm, 0.0)
            for _ in range(40):
                nc.tensor.ldweights(weights=warm)
            emit_qk(0, 0, 0)
            emit_qk(1, 0, 0)
            # deferred init ops (off the startup critical path)
            ident = const.tile([128, 128], f16)
            make_identity(nc, ident)
            ones_cols = Vaug.rearrange("p pp J (h x) -> p pp J h x",
                                       x=65)[:, :, :, :, 64]
            nc.gpsimd.memset(ones_cols, 1.0)
            emit_qk(2, 0, 0)
            emit_qk(3, 0, 0)
            emit_qk(0, 0, 1)

            # ---------- windowed attention schedule ----------
            # Score+exp windows (1024-wide) pace the scalar engine; one PE
            # filler granule (~1.5-2us) is emitted per window so the PE never
            # idles long enough to cold-throttle. PV/norm/proj granules are
            # scheduled a couple of windows after their last input exp.
            fillers = [
                lambda: emit_qk(1, 0, 1),        # K-p0 c1 (scores j>=4)
                lambda: emit_v(0, 0),            # PV-p0 c0
                lambda: emit_qk(2, 0, 1),        # K-p0 c2 (scores j>=8)
                lambda: emit_qk(0, 1, 0),
                lambda: emit_qk(3, 0, 1),        # K-p0 c3 (scores j>=12)
                lambda: emit_v(1, 0),            # PV-p0 c1
                lambda: emit_qk(0, 1, 1),
                lambda: emit_qk(1, 1, 0),
                lambda: emit_v(2, 0),            # PV-p0 c2
                lambda: emit_qk(1, 1, 1),
                lambda: emit_qk(2, 1, 0),
                lambda: emit_v(3, 0),            # PV-p0 c3
                lambda: emit_qk(2, 1, 1),
                lambda: emit_qk(3, 1, 0),
                lambda: emit_qk(3, 1, 1),
            ]
            fillers_p1 = [
                lambda: emit_v(0, 1),            # PV-p1 c0
                lambda: emit_v(1, 1),
                lambda: emit_v(2, 1),
                lambda: emit_v(3, 1),
            ]
            yps_box = [{}, {}]  # per pair: h2 -> yps tile

            def mk_pv(p, c, h2):
                def f():
                    emit_pv_head(p, c, h2, yps_box[p])
                    emit_norm(p, c, h2, yps_box[p][h2])
                return f

            def mk_proj(tt2):
                return lambda: emit_proj(tt2)

            for p in range(NP):
                # per-pair window list with last-window index per chunk
                windows = []
                for j in range(NKB):
                    L = T - j * KB
                    for w0 in range(0, L, 1024):
                        windows.append((j, w0, L))
                ready_at = {}
                for wi, (j, w0, L) in enumerate(windows):
                    if j % 4 == 3 and w0 + 1024 >= L:
                        ready_at[j // 4] = wi
                # PV/norm h2=0 two windows after the chunk's last exp,
                # h2=1 one window later; projection (pair 1) after that.
                # Chunk 3 flushes right after the final window of the pair.
                sched = {}
                for c, wi in ready_at.items():
                    d = 1 if c < 3 else 0
                    sched.setdefault(wi + d, []).append(mk_pv(p, c, 0))
                    sched.setdefault(wi + d + (1 if c < 3 else 0),
                                     []).append(mk_pv(p, c, 1))
                    if p == 1:
                        sched.setdefault(wi + d + 1, []).append(
                            mk_proj(2 * c))
                        sched.setdefault(wi + d + 2, []).append(
                            mk_proj(2 * c + 1))

                nwin = len(windows)
                fq = fillers if p == 0 else fillers_p1
                for wi, (j, w0, L) in enumerate(windows):
                    emit_score_window(p, j, w0)
                    for f in sched.pop(wi, []):
                        f()
                    if fq and (p == 0 or wi % 2 == 0):
                        fq.pop(0)()
                # flush anything scheduled past the last window
                for wi in sorted(k for k in sched if k >= nwin):
                    for f in sched.pop(wi):
                        f()
                assert not sched, sched
                for f in fq:
                    f()
                fq.clear()

    nc.finalize()
    return nc


def _host_prep(x, cos, sin, w_attn, b_attn, w_proj):
    """Shared + per-core input arrays (all fp16)."""
    x = np.asarray(x, dtype=np.float32)
    xT16 = [np.ascontiguousarray(x[b].T).astype(np.float16) for b in range(B)]

    cos = np.asarray(cos, dtype=np.float32)
    sin = np.asarray(sin, dtype=np.float32)
    d = np.arange(128) % 64
    freq_i = d // 2
    sign = np.where(d % 2 == 0, -1.0, 1.0).astype(np.float32)
    cos_exp = cos[:, freq_i].T.astype(np.float16)            # [128, T]
    sin_exp = (sign[:, None] * sin[:, freq_i].T).astype(np.float16)

    pswap = np.zeros((128, 128), dtype=np.float16)
    idx = np.arange(128)
    pswap[idx ^ 1, idx] = 1.0

    # [k, q] = -100 where q < k (pre-exp additive causal mask)
    mneg = np.where(np.arange(128)[None, :] < np.arange(128)[:, None],
                    np.float16(-100.0), np.float16(0.0)).astype(np.float16)

    w_attn = np.asarray(w_attn, dtype=np.float32)
    w_proj = np.asarray(w_proj, dtype=np.float32)
    scale = 1.0 / np.sqrt(HD)

    per_core = []
    for m in range(N_CORES):
        hg = m % TPC
        heads = [4 * hg + k for k in range(4)]
        cols = []
        # groups: Q-p0, K-p0, Q-p1, K-p1, V-p0, V-p1 (heads pair-major)
        for p in range(NP):
            for g in range(2):  # Q, K
                for hh in heads[2 * p:2 * p + 2]:
                    blk = w_attn[:, g * C + hh * HD: g * C + (hh + 1) * HD]
                    if g == 0:
                        blk = blk * scale
                    cols.append(blk)
        for p in range(NP):
            for hh in heads[2 * p:2 * p + 2]:
                cols.append(w_attn[:, 2 * C + hh * HD: 2 * C + (hh + 1) * HD])
        w_stack = np.concatenate(cols, axis=1).astype(np.float16)
        wp_m = np.concatenate(
            [w_proj[hh * HD:(hh + 1) * HD, :] for hh in heads],
            axis=0).astype(np.float16)
        per_core.append((w_stack, wp_m))
    return xT16, cos_exp, sin_exp, pswap, mneg, per_core


def _build_in_maps(inputs):
    xT16, cos_exp, sin_exp, pswap, mneg, per_core = _host_prep(
        inputs["x"], inputs["cos"], inputs["sin"], inputs["w_attn"],
        inputs["b_attn"], inputs["w_proj"])
    in_maps = []
    for m in range(N_CORES):
        w_stack, wp_m = per_core[m]
        in_maps.append({
            "xT": xT16[m // TPC], "wqkv": w_stack, "wp": wp_m,
            "cos_d": cos_exp, "sin_d": sin_exp, "pswap": pswap,
            "maskneg": mneg,
        })
    return in_maps


def kernel(x, cos, sin, w_attn, b_attn, w_proj, b_proj):
    from concourse.bass_utils import run_bass_kernel_spmd

    b_attn = np.asarray(b_attn, dtype=np.float32)
    assert not np.any(b_attn), "nonzero b_attn not supported by this kernel"

    in_maps = _build_in_maps({
        "x": x, "cos": cos, "sin": sin, "w_attn": w_attn,
        "b_attn": b_attn, "w_proj": w_proj})

    if "nc" not in _CACHE:
        _CACHE["nc"] = _build_bass()
    nc = _CACHE["nc"]

    res = run_bass_kernel_spmd(nc, in_maps, core_ids=list(range(N_CORES)))
    _CACHE["last_result"] = res

    yf = np.zeros((B, T, C), dtype=np.float64)
    for m in range(N_CORES):
        yf[m // TPC] += res.results[m]["y"].astype(np.float64)
    yf = yf + np.asarray(b_proj, dtype=np.float64)[None, None, :]
    return yf.astype(np.float32)


# revision 27
# speedup vs baseline: 1.0005x; 1.0005x over previous
"""Causal self-attention with rotary embeddings on 8 Trainium2 NeuronCores.

v2: 2-way batch data-parallel x 4-way head tensor-parallel.
Core m handles batch m//4 and heads 4*(m%4)..4*(m%4)+3 (two pairs).
Each core computes qkv for its 4 heads, rotary, causal attention, and a
partial output projection (its 256 rows of w_proj) over its batch's
2048 tokens; the host sums 4 partials per batch.

Device-side layout (per core; heads within a pair at partitions 0-63 / 64-127):
  - Everything "transposed": Q^T/K^T stored [d(128), pair, t(2048)].
  - Scores S^T = K_blk @ Q^T -> [k(128), q]; the two heads of a pair issue
    back-to-back K=64 matmuls into different PE row groups (tile_position
    (0,0)/(64,0)) so they run concurrently.
  - exp on the scalar engine (PSUM fp32 -> SBUF fp16 pt tiles); softmax's
    k-sum folded into P@V via a ones-augmented V column (denominator row).
  - Rotary via pair-swap permutation matmul: rot(q) = cos*q + sin_sgn*(Pswap@q).
  - V transposed to t-major via hardware DMA transpose (XBAR), not the PE.
  - PV accumulates the causal triangle with narrow diagonal-block matmuls
    (has_written semantics preserve untouched columns); no zero-padding.
  - Emission interleaves pair-1 QK, V-groups and PV/projection chunks into
    the exp-paced score stream so the PE never head-of-line blocks.
"""

import numpy as np

B, T, C, H = 2, 2048, 1024, 16
HD = C // H            # 64
N_CORES = 8
DP = 2                 # batch shards
TPC = 4                # head-group shards
NP = 2                 # pairs per core (4 heads)
TC = 512               # t-chunk
NTC = T // TC          # 4
KB = 128               # k-block
NKB = T // KB          # 16
QC = 512               # q-chunk for PV
NQC = T // QC          # 4

_CACHE = {}


def _build_bass():
    import concourse.bacc as bacc
    import concourse.mybir as mybir
    import concourse.tile as tile
    from concourse.masks import make_identity

    f16 = mybir.dt.float16
    f32 = mybir.dt.float32

    nc = bacc.Bacc()

    xT = nc.dram_tensor("xT", [C, T], f16, kind="ExternalInput")
    wqkv = nc.dram_tensor("wqkv", [C, 6 * 128], f16, kind="ExternalInput")
    wp = nc.dram_tensor("wp", [2 * 128, C], f16, kind="ExternalInput")
    cos_d = nc.dram_tensor("cos_d", [128, T], f16, kind="ExternalInput")
    sin_d = nc.dram_tensor("sin_d", [128, T], f16, kind="ExternalInput")
    pswap = nc.dram_tensor("pswap", [128, 128], f16, kind="ExternalInput")
    maskneg = nc.dram_tensor("maskneg", [128, 128], f16, kind="ExternalInput")
    y = nc.dram_tensor("y", [T, C], f16, kind="ExternalOutput")

    CCH = C // 128  # 8 contraction chunks

    with tile.TileContext(nc) as tc:
        with (
            tc.tile_pool(name="const", bufs=1) as const,
            tc.tile_pool(name="persist", bufs=1) as persist,
            tc.tile_pool(name="ptp", bufs=1) as ptp,
            tc.tile_pool(name="stream", bufs=2) as stream,
            tc.tile_pool(name="psum", bufs=1, space="PSUM") as psum,
        ):
            # ---- constants (ordered so the first QKV group starts ASAP) ----
            xT_r = xT.rearrange("(cc p) t -> p cc t", p=128)
            x_all = persist.tile([128, CCH, NTC, TC], f16)
            nc.sync.dma_start(out=x_all[:, :, 0, :], in_=xT_r[:, :, 0:TC])
            wqkv_sb = const.tile([128, CCH, 6, 128], f16)
            wqkv_r = wqkv.rearrange("(cc p) (g j) -> p cc g j", p=128, j=128)
            nc.sync.dma_start(out=wqkv_sb[:, :, 0, :], in_=wqkv_r[:, :, 0, :])
            nc.sync.dma_start(out=wqkv_sb[:, :, 1, :], in_=wqkv_r[:, :, 1, :])
            pswap_sb = const.tile([128, 128], f16)
            nc.sync.dma_start(out=pswap_sb, in_=pswap[:, :])
            cos_sb = const.tile([128, T], f16)
            nc.sync.dma_start(out=cos_sb, in_=cos_d[:, :])
            sin_sb = const.tile([128, T], f16)
            nc.sync.dma_start(out=sin_sb, in_=sin_d[:, :])
            for i in range(1, NTC):
                nc.sync.dma_start(out=x_all[:, :, i, :],
                                  in_=xT_r[:, :, i * TC:(i + 1) * TC])
            nc.sync.dma_start(out=wqkv_sb[:, :, 2:6, :],
                              in_=wqkv_r[:, :, 2:6, :])
            wp_sb = const.tile([128, 2, C], f16)
            nc.sync.dma_start(out=wp_sb, in_=wp.rearrange("(pp p) c -> p pp c",
                                                          p=128))
            # maskneg[k, q] = -100 where q < k, else 0 (added to diag scores
            # pre-exp via an identity matmul, so masking costs PE only)
            mneg_sb = const.tile([128, 128], f16)
            nc.sync.dma_start(out=mneg_sb, in_=maskneg[:, :])

            # ---- persistent tensors ----
            QrotT = persist.tile([128, NP, T], f16)
            KrotT = persist.tile([128, NP, T], f16)
            # V t-major per (pair, k-block): [V_A(64) | ones | V_B(64) | ones]
            Vaug = persist.tile([128, NP, NKB, 130], f16)
            Yn = persist.tile([128, NP, T], f16)

            # ---------- emission helpers ----------
            def emit_qk(i, p, gk):
                """QKV group for (chunk i, pair p, Q:gk=0/K:gk=1) + rotary."""
                ts = slice(i * TC, (i + 1) * TC)
                g = 2 * p + gk
                dst = QrotT if gk == 0 else KrotT
                acc = psum.tile([128, TC], f32, tag="acc", bufs=4, name="acc")
                for cc in range(CCH):
                    nc.tensor.matmul(
                        acc, wqkv_sb[:, cc, g, :], x_all[:, cc, i, :],
                        start=(cc == 0), stop=(cc == CCH - 1))
                graw = stream.tile([128, TC], f16, tag="graw", bufs=3)
                nc.vector.tensor_copy(graw, acc)
                swp = psum.tile([128, TC], f32, tag="acc", bufs=4, name="swp")
                nc.tensor.matmul(swp, pswap_sb, graw, start=True, stop=True)
                t1 = stream.tile([128, TC], f16, tag="t1")
                nc.vector.tensor_mul(t1, graw, cos_sb[:, ts])
                t2 = stream.tile([128, TC], f16, tag="t2")
                nc.vector.tensor_mul(t2, swp, sin_sb[:, ts])
                nc.vector.tensor_add(dst[:, p, ts], t1, t2)

            def emit_v(i, p):
                """V group for (chunk i, pair p): matmul + DMA transpose."""
                g = 4 + p
                acc = psum.tile([128, TC], f32, tag="acc", bufs=4, name="vacc")
                for cc in range(CCH):
                    nc.tensor.matmul(
                        acc, wqkv_sb[:, cc, g, :], x_all[:, cc, i, :],
                        start=(cc == 0), stop=(cc == CCH - 1))
                vtmp = stream.tile([128, TC], f16, tag="vtmp")
                nc.vector.tensor_copy(vtmp, acc)
                vt4 = stream.tile([128, 4, 128], f16, tag="vt4")
                nc.sync.dma_start_transpose(out=vt4[:, :, :], in_=vtmp)
                # vt4[tlo, thi, d]; d 0-63 head A, 64-127 head B
                Vr = Vaug.rearrange("p pp J (h x) -> p pp J h x", x=65)
                nc.vector.tensor_copy(
                    Vr[:, p, 4 * i:4 * i + 4, 0, 0:64], vt4[:, :, 0:64])
                nc.vector.tensor_copy(
                    Vr[:, p, 4 * i:4 * i + 4, 1, 0:64], vt4[:, :, 64:128])

            pt = {}  # (pair-local head h2, j) -> tile

            def emit_score_window(p, j, w0):
                """One 1024-wide score+exp window for both heads of pair p."""
                L = T - j * KB
                k0 = j * KB
                if w0 == 0:
                    ptA = ptp.tile([128, L], f16, tag=f"pt0_{j}", bufs=1,
                                   name="ptA")
                    ptB = ptp.tile([128, L], f16, tag=f"pt1_{j}", bufs=1,
                                   name="ptB")
                    pt[(0, j)] = ptA
                    pt[(1, j)] = ptB
                ptA, ptB = pt[(0, j)], pt[(1, j)]
                nw = min(1024, L - w0)
                stA = psum.tile([128, 1024], f32, tag="stA", bufs=1,
                                name="stA")
                stB = psum.tile([128, 1024], f32, tag="stB", bufs=1,
                                name="stB")
                for s0 in range(0, nw, 512):
                    ns = min(512, nw - s0)
                    q0 = k0 + w0 + s0
                    diag = (w0 == 0 and s0 == 0)
                    nc.tensor.matmul(
                        stA[:, s0:s0 + ns], KrotT[0:64, p, k0:k0 + 128],
                        QrotT[0:64, p, q0:q0 + ns], start=True,
                        stop=not diag)
                    nc.tensor.matmul(
                        stB[:, s0:s0 + ns], KrotT[64:128, p, k0:k0 + 128],
                        QrotT[64:128, p, q0:q0 + ns], start=True,
                        stop=not diag)
                    if diag:
                        # add -100 above the causal diagonal before exp
                        nc.tensor.matmul(stA[:, 0:128], ident, mneg_sb,
                                         start=False, stop=True)
                        nc.tensor.matmul(stB[:, 0:128], ident, mneg_sb,
                                         start=False, stop=True)
                nc.scalar.activation(
                    ptA[:, w0:w0 + nw], stA[:, 0:nw],
                    mybir.ActivationFunctionType.Exp)
                nc.scalar.activation(
                    ptB[:, w0:w0 + nw], stB[:, 0:nw],
                    mybir.ActivationFunctionType.Exp)

            def emit_pv_head(p, c, h2, yps_out):
                """PV accumulation for one head of one q-chunk (<=16 MMs)."""
                jmax = 4 * c + 3
                ypst = psum.tile([128, QC], f32, tag="acc", bufs=4,
                                 name="yps")
                yps_out[h2] = ypst
                for j in range(jmax + 1):
                    lhsT = Vaug[:, p, j, h2 * 65:(h2 + 1) * 65]
                    off = c * QC - j * KB
                    if off >= 0:
                        rhs = pt[(h2, j)][:, off:off + QC]
                        out = ypst[0:65, :]
                    else:
                        rhs = pt[(h2, j)][:, 0:QC + off]
                        out = ypst[0:65, -off:QC]
                    nc.tensor.matmul(out, lhsT, rhs,
                                     start=(j == 0), stop=(j == jmax))

            def emit_norm(p, c, h2, ypst):
                """normalize rows 0-63 by the ones-row (64) -> Yn.
                Broadcast of 1/den via a K=1 PE matmul so gpsimd's FIFO
                (which runs the causal masks) is never on this chain."""
                dsb = stream.tile([128, QC], f32, tag="dsb")
                nc.vector.tensor_copy(dsb[0:1, :], ypst[64:65, :])
                recip = stream.tile([128, QC], f32, tag="recip")
                nc.vector.reciprocal_approx_fast(
                    out=recip[0:1, :], in_=dsb[0:1, :])
                bc = stream.tile([128, QC], f32, tag="bc")
                nc.gpsimd.partition_broadcast(bc[0:64, :], recip[0:1, :])
                if h2 == 0:
                    nc.vector.tensor_tensor(
                        out=Yn[0:64, p, c * QC:(c + 1) * QC],
                        in0=ypst[0:64, :], in1=bc[0:64, :],
                        op=mybir.AluOpType.mult)
                else:
                    ytmp = stream.tile([128, QC], f16, tag="ytmp")
                    nc.vector.tensor_tensor(
                        out=ytmp[0:64, :], in0=ypst[0:64, :],
                        in1=bc[0:64, :], op=mybir.AluOpType.mult)
                    nc.sync.dma_start(
                        out=Yn[64:128, p, c * QC:(c + 1) * QC],
                        in_=ytmp[0:64, :])

            def emit_proj(tt2):
                """Projection for a 256-token block (4 MMs, 2 evacs)."""
                for tt in range(2 * tt2, 2 * tt2 + 2):
                    for half in range(2):
                        pout = psum.tile([128, 512], f32, tag="acc",
                                         bufs=4, name="pout")
                        for pp in range(2):
                            nc.tensor.matmul(
                                pout, Yn[:, pp, tt * 128:(tt + 1) * 128],
                                wp_sb[:, pp, half * 512:(half + 1) * 512],
                                start=(pp == 0), stop=(pp == 1))
                        yout = stream.tile([128, 512], f16, tag="yo")
                        nc.vector.tensor_copy(yout, pout)
                        nc.sync.dma_start(
                            out=y[tt * 128:(tt + 1) * 128,
                                  half * 512:(half + 1) * 512],
                            in_=yout)

            # ---------- phase 1 lead-in: pair-0 Q + first K ----------
            emit_qk(0, 0, 0)
            emit_qk(1, 0, 0)
            # deferred init ops (off the startup critical path)
            ident = const.tile([128, 128], f16)
            make_identity(nc, ident)
            ones_cols = Vaug.rearrange("p pp J (h x) -> p pp J h x",
                                       x=65)[:, :, :, :, 64]
            nc.gpsimd.memset(ones_cols, 1.0)
            emit_qk(2, 0, 0)
            emit_qk(3, 0, 0)
            emit_qk(0, 0, 1)

            # ---------- windowed attention schedule ----------
            # Score+exp windows (1024-wide) pace the scalar engine; one PE
            # filler granule (~1.5-2us) is emitted per window so the PE never
            # idles long enough to cold-throttle. PV/norm/proj granules are
            # scheduled a couple of windows after their last input exp.
            fillers = [
                lambda: emit_qk(1, 0, 1),        # K-p0 c1 (scores j>=4)
                lambda: emit_v(0, 0),            # PV-p0 c0
                lambda: emit_qk(2, 0, 1),        # K-p0 c2 (scores j>=8)
                lambda: emit_qk(0, 1, 0),
                lambda: emit_qk(3, 0, 1),        # K-p0 c3 (scores j>=12)
                lambda: emit_v(1, 0),            # PV-p0 c1
                lambda: emit_qk(0, 1, 1),
                lambda: emit_qk(1, 1, 0),
                lambda: emit_v(2, 0),            # PV-p0 c2
                lambda: emit_qk(1, 1, 1),
                lambda: emit_qk(2, 1, 0),
                lambda: emit_v(3, 0),            # PV-p0 c3
                lambda: emit_qk(2, 1, 1),
                lambda: emit_qk(3, 1, 0),
                lambda: emit_qk(3, 1, 1),
            ]
            fillers_p1 = [
                lambda: emit_v(0, 1),            # PV-p1 c0
                lambda: emit_v(1, 1),
                lambda: emit_v(2, 1),
                lambda: emit_v(3, 1),
            ]
            yps_box = [{}, {}]  # per pair: h2 -> yps tile

            def mk_pv(p, c, h2):
                def f():
                    emit_pv_head(p, c, h2, yps_box[p])
                    emit_norm(p, c, h2, yps_box[p][h2])
                return f

            def mk_proj(tt2):
                return lambda: emit_proj(tt2)

            for p in range(NP):
                # per-pair window list with last-window index per chunk
                windows = []
                for j in range(NKB):
                    L = T - j * KB
                    for w0 in range(0, L, 1024):
                        windows.append((j, w0, L))
                ready_at = {}
                for wi, (j, w0, L) in enumerate(windows):
                    if j % 4 == 3 and w0 + 1024 >= L:
                        ready_at[j // 4] = wi
                # PV/norm h2=0 two windows after the chunk's last exp,
                # h2=1 one window later; projection (pair 1) after that.
                # Chunk 3 flushes right after the final window of the pair.
                sched = {}
                for c, wi in ready_at.items():
                    d = 1 if c < 3 else 0
                    sched.setdefault(wi + d, []).append(mk_pv(p, c, 0))
                    sched.setdefault(wi + d + (1 if c < 3 else 0),
                                     []).append(mk_pv(p, c, 1))
                    if p == 1:
                        sched.setdefault(wi + d + 1, []).append(
                            mk_proj(2 * c))
                        sched.setdefault(wi + d + 2, []).append(
                            mk_proj(2 * c + 1))

                nwin = len(windows)
                fq = fillers if p == 0 else fillers_p1
                for wi, (j, w0, L) in enumerate(windows):
                    emit_score_window(p, j, w0)
                    for f in sched.pop(wi, []):
                        f()
                    if fq and (p == 0 or wi % 2 == 0):
                        fq.pop(0)()
                # flush anything scheduled past the last window
                for wi in sorted(k for k in sched if k >= nwin):
                    for f in sched.pop(wi):
                        f()
                assert not sched, sched
                for f in fq:
                    f()
                fq.clear()

    nc.finalize()
    return nc


def _host_prep(x, cos, sin, w_attn, b_attn, w_proj):
    """Shared + per-core input arrays (all fp16)."""
    x = np.asarray(x, dtype=np.float32)
    xT16 = [np.ascontiguousarray(x[b].T).astype(np.float16) for b in range(B)]

    cos = np.asarray(cos, dtype=np.float32)
    sin = np.asarray(sin, dtype=np.float32)
    d = np.arange(128) % 64
    freq_i = d // 2
    sign = np.where(d % 2 == 0, -1.0, 1.0).astype(np.float32)
    cos_exp = cos[:, freq_i].T.astype(np.float16)            # [128, T]
    sin_exp = (sign[:, None] * sin[:, freq_i].T).astype(np.float16)

    pswap = np.zeros((128, 128), dtype=np.float16)
    idx = np.arange(128)
    pswap[idx ^ 1, idx] = 1.0

    # [k, q] = -100 where q < k (pre-exp additive causal mask)
    mneg = np.where(np.arange(128)[None, :] < np.arange(128)[:, None],
                    np.float16(-100.0), np.float16(0.0)).astype(np.float16)

    w_attn = np.asarray(w_attn, dtype=np.float32)
    w_proj = np.asarray(w_proj, dtype=np.float32)
    scale = 1.0 / np.sqrt(HD)

    per_core = []
    for m in range(N_CORES):
        hg = m % TPC
        heads = [4 * hg + k for k in range(4)]
        cols = []
        # groups: Q-p0, K-p0, Q-p1, K-p1, V-p0, V-p1 (heads pair-major)
        for p in range(NP):
            for g in range(2):  # Q, K
                for hh in heads[2 * p:2 * p + 2]:
                    blk = w_attn[:, g * C + hh * HD: g * C + (hh + 1) * HD]
                    if g == 0:
                        blk = blk * scale
                    cols.append(blk)
        for p in range(NP):
            for hh in heads[2 * p:2 * p + 2]:
                cols.append(w_attn[:, 2 * C + hh * HD: 2 * C + (hh + 1) * HD])
        w_stack = np.concatenate(cols, axis=1).astype(np.float16)
        wp_m = np.concatenate(
            [w_proj[hh * HD:(hh + 1) * HD, :] for hh in heads],
            axis=0).astype(np.float16)
        per_core.append((w_stack, wp_m))
    return xT16, cos_exp, sin_exp, pswap, mneg, per_core


def _build_in_maps(inputs):
    xT16, cos_exp, sin_exp, pswap, mneg, per_core = _host_prep(
        inputs["x"], inputs["cos"], inputs["sin"], inputs["w_attn"],
        inputs["b_attn"], inputs["w_proj"])
    in_maps = []
    for m in range(N_CORES):
        w_stack, wp_m = per_core[m]
        in_maps.append({
            "xT": xT16[m // TPC], "wqkv": w_stack, "wp": wp_m,
            "cos_d": cos_exp, "sin_d": sin_exp, "pswap": pswap,
            "maskneg": mneg,
        })
    return in_maps


def kernel(x, cos, sin, w_attn, b_attn, w_proj, b_proj):
    from concourse.bass_utils import run_bass_kernel_spmd

    b_attn = np.asarray(b_attn, dtype=np.float32)
    assert not np.any(b_attn), "nonzero b_attn not supported by this kernel"

    in_maps = _build_in_maps({
        "x": x, "cos": cos, "sin": sin, "w_attn": w_attn,
        "b_attn": b_attn, "w_proj": w_proj})

    if "nc" not in _CACHE:
        _CACHE["nc"] = _build_bass()
    nc = _CACHE["nc"]

    res = run_bass_kernel_spmd(nc, in_maps, core_ids=list(range(N_CORES)))
    _CACHE["last_result"] = res

    yf = np.zeros((B, T, C), dtype=np.float64)
    for m in range(N_CORES):
        yf[m // TPC] += res.results[m]["y"].astype(np.float64)
    yf = yf + np.asarray(b_proj, dtype=np.float64)[None, None, :]
    return yf.astype(np.float32)


# revision 32
# speedup vs baseline: 1.0115x; 1.0111x over previous
"""Causal self-attention with rotary embeddings on 8 Trainium2 NeuronCores.

v2: 2-way batch data-parallel x 4-way head tensor-parallel.
Core m handles batch m//4 and heads 4*(m%4)..4*(m%4)+3 (two pairs).
Each core computes qkv for its 4 heads, rotary, causal attention, and a
partial output projection (its 256 rows of w_proj) over its batch's
2048 tokens; the host sums 4 partials per batch.

Device-side layout (per core; heads within a pair at partitions 0-63 / 64-127):
  - Everything "transposed": Q^T/K^T stored [d(128), pair, t(2048)].
  - Scores S^T = K_blk @ Q^T -> [k(128), q]; the two heads of a pair issue
    back-to-back K=64 matmuls into different PE row groups (tile_position
    (0,0)/(64,0)) so they run concurrently.
  - exp on the scalar engine (PSUM fp32 -> SBUF fp16 pt tiles); softmax's
    k-sum folded into P@V via a ones-augmented V column (denominator row).
  - Rotary via pair-swap permutation matmul: rot(q) = cos*q + sin_sgn*(Pswap@q).
  - V transposed to t-major via hardware DMA transpose (XBAR), not the PE.
  - PV accumulates the causal triangle with narrow diagonal-block matmuls
    (has_written semantics preserve untouched columns); no zero-padding.
  - Emission interleaves pair-1 QK, V-groups and PV/projection chunks into
    the exp-paced score stream so the PE never head-of-line blocks.
"""

import numpy as np

B, T, C, H = 2, 2048, 1024, 16
HD = C // H            # 64
N_CORES = 8
DP = 2                 # batch shards
TPC = 4                # head-group shards
NP = 2                 # pairs per core (4 heads)
TC = 512               # t-chunk
NTC = T // TC          # 4
KB = 128               # k-block
NKB = T // KB          # 16
QC = 512               # q-chunk for PV
NQC = T // QC          # 4

_CACHE = {}


def _build_bass():
    import concourse.bacc as bacc
    import concourse.mybir as mybir
    import concourse.tile as tile
    from concourse.masks import make_identity

    f16 = mybir.dt.float16
    f32 = mybir.dt.float32

    nc = bacc.Bacc()

    xT = nc.dram_tensor("xT", [C, T], f16, kind="ExternalInput")
    wqkv = nc.dram_tensor("wqkv", [C, 6 * 128], f16, kind="ExternalInput")
    wp = nc.dram_tensor("wp", [2 * 128, C], f16, kind="ExternalInput")
    cos_d = nc.dram_tensor("cos_d", [128, T], f16, kind="ExternalInput")
    sin_d = nc.dram_tensor("sin_d", [128, T], f16, kind="ExternalInput")
    pswap = nc.dram_tensor("pswap", [128, 128], f16, kind="ExternalInput")
    maskneg = nc.dram_tensor("maskneg", [128, 128], f16, kind="ExternalInput")
    y = nc.dram_tensor("y", [T, C], f16, kind="ExternalOutput")

    CCH = C // 128  # 8 contraction chunks

    with tile.TileContext(nc) as tc:
        with (
            tc.tile_pool(name="const", bufs=1) as const,
            tc.tile_pool(name="persist", bufs=1) as persist,
            tc.tile_pool(name="ptp", bufs=1) as ptp,
            tc.tile_pool(name="stream", bufs=2) as stream,
            tc.tile_pool(name="psum", bufs=1, space="PSUM") as psum,
        ):
            # ---- constants (ordered so the first QKV group starts ASAP) ----
            xT_r = xT.rearrange("(cc p) t -> p cc t", p=128)
            x_all = persist.tile([128, CCH, NTC, TC], f16)
            nc.sync.dma_start(out=x_all[:, :, 0, :], in_=xT_r[:, :, 0:TC])
            wqkv_sb = const.tile([128, CCH, 6, 128], f16)
            wqkv_r = wqkv.rearrange("(cc p) (g j) -> p cc g j", p=128, j=128)
            nc.sync.dma_start(out=wqkv_sb[:, :, 0, :], in_=wqkv_r[:, :, 0, :])
            nc.sync.dma_start(out=wqkv_sb[:, :, 1, :], in_=wqkv_r[:, :, 1, :])
            nc.sync.dma_start(out=x_all[:, :, 1, :], in_=xT_r[:, :, TC:2 * TC])
            pswap_sb = const.tile([128, 128], f16)
            nc.sync.dma_start(out=pswap_sb, in_=pswap[:, :])
            cos_sb = const.tile([128, T], f16)
            nc.sync.dma_start(out=cos_sb, in_=cos_d[:, :])
            sin_sb = const.tile([128, T], f16)
            nc.sync.dma_start(out=sin_sb, in_=sin_d[:, :])
            for i in range(2, NTC):
                nc.sync.dma_start(out=x_all[:, :, i, :],
                                  in_=xT_r[:, :, i * TC:(i + 1) * TC])
            nc.sync.dma_start(out=wqkv_sb[:, :, 2:6, :],
                              in_=wqkv_r[:, :, 2:6, :])
            wp_sb = const.tile([128, 2, C], f16)
            nc.sync.dma_start(out=wp_sb, in_=wp.rearrange("(pp p) c -> p pp c",
                                                          p=128))
            # maskneg[k, q] = -100 where q < k, else 0 (added to diag scores
            # pre-exp via an identity matmul, so masking costs PE only)
            mneg_sb = const.tile([128, 128], f16)
            nc.sync.dma_start(out=mneg_sb, in_=maskneg[:, :])

            # ---- persistent tensors ----
            QrotT = persist.tile([128, NP, T], f16)
            KrotT = persist.tile([128, NP, T], f16)
            # V t-major per (pair, k-block): [V_A(64) | ones | V_B(64) | ones]
            Vaug = persist.tile([128, NP, NKB, 130], f16)
            Yn = persist.tile([128, NP, T], f16)

            # ---------- emission helpers ----------
            def emit_qk(i, p, gk):
                """QKV group for (chunk i, pair p, Q:gk=0/K:gk=1) + rotary."""
                ts = slice(i * TC, (i + 1) * TC)
                g = 2 * p + gk
                dst = QrotT if gk == 0 else KrotT
                acc = psum.tile([128, TC], f32, tag="acc", bufs=4, name="acc")
                for cc in range(CCH):
                    nc.tensor.matmul(
                        acc, wqkv_sb[:, cc, g, :], x_all[:, cc, i, :],
                        start=(cc == 0), stop=(cc == CCH - 1))
                graw = stream.tile([128, TC], f16, tag="graw", bufs=3)
                nc.vector.tensor_copy(graw, acc)
                swp = psum.tile([128, TC], f32, tag="acc", bufs=4, name="swp")
                nc.tensor.matmul(swp, pswap_sb, graw, start=True, stop=True)
                t1 = stream.tile([128, TC], f16, tag="t1")
                nc.vector.tensor_mul(t1, graw, cos_sb[:, ts])
                t2 = stream.tile([128, TC], f16, tag="t2")
                nc.vector.tensor_mul(t2, swp, sin_sb[:, ts])
                nc.vector.tensor_add(dst[:, p, ts], t1, t2)

            def emit_v(i, p):
                """V group for (chunk i, pair p): matmul + DMA transpose."""
                g = 4 + p
                acc = psum.tile([128, TC], f32, tag="acc", bufs=4, name="vacc")
                for cc in range(CCH):
                    nc.tensor.matmul(
                        acc, wqkv_sb[:, cc, g, :], x_all[:, cc, i, :],
                        start=(cc == 0), stop=(cc == CCH - 1))
                vtmp = stream.tile([128, TC], f16, tag="vtmp")
                nc.vector.tensor_copy(vtmp, acc)
                vt4 = stream.tile([128, 4, 128], f16, tag="vt4")
                nc.sync.dma_start_transpose(out=vt4[:, :, :], in_=vtmp)
                # vt4[tlo, thi, d]; d 0-63 head A, 64-127 head B
                Vr = Vaug.rearrange("p pp J (h x) -> p pp J h x", x=65)
                nc.vector.tensor_copy(
                    Vr[:, p, 4 * i:4 * i + 4, 0, 0:64], vt4[:, :, 0:64])
                nc.vector.tensor_copy(
                    Vr[:, p, 4 * i:4 * i + 4, 1, 0:64], vt4[:, :, 64:128])

            # pt tiles per pair (the tag ring shares buffers across pairs;
            # keeping separate dicts preserves each pair's dependency chain)
            pt = [{}, {}]

            def emit_score_window(p, j, w0):
                """One 1024-wide score+exp window for both heads of pair p."""
                L = T - j * KB
                k0 = j * KB
                if w0 == 0:
                    ptA = ptp.tile([128, L], f16, tag=f"pt0_{j}", bufs=1,
                                   name="ptA")
                    ptB = ptp.tile([128, L], f16, tag=f"pt1_{j}", bufs=1,
                                   name="ptB")
                    pt[p][(0, j)] = ptA
                    pt[p][(1, j)] = ptB
                ptA, ptB = pt[p][(0, j)], pt[p][(1, j)]
                nw = min(1024, L - w0)
                stA = psum.tile([128, 1024], f32, tag="stA", bufs=1,
                                name="stA")
                stB = psum.tile([128, 1024], f32, tag="stB", bufs=1,
                                name="stB")
                for s0 in range(0, nw, 512):
                    ns = min(512, nw - s0)
                    q0 = k0 + w0 + s0
                    diag = (w0 == 0 and s0 == 0)
                    nc.tensor.matmul(
                        stA[:, s0:s0 + ns], KrotT[0:64, p, k0:k0 + 128],
                        QrotT[0:64, p, q0:q0 + ns], start=True,
                        stop=not diag)
                    nc.tensor.matmul(
                        stB[:, s0:s0 + ns], KrotT[64:128, p, k0:k0 + 128],
                        QrotT[64:128, p, q0:q0 + ns], start=True,
                        stop=not diag)
                    if diag:
                        # add -100 above the causal diagonal before exp
                        nc.tensor.matmul(stA[:, 0:128], ident, mneg_sb,
                                         start=False, stop=True)
                        nc.tensor.matmul(stB[:, 0:128], ident, mneg_sb,
                                         start=False, stop=True)
                nc.scalar.activation(
                    ptA[:, w0:w0 + nw], stA[:, 0:nw],
                    mybir.ActivationFunctionType.Exp)
                nc.scalar.activation(
                    ptB[:, w0:w0 + nw], stB[:, 0:nw],
                    mybir.ActivationFunctionType.Exp)

            def emit_pv_head(p, c, h2, yps_out):
                """PV accumulation for one head of one q-chunk (<=16 MMs)."""
                jmax = 4 * c + 3
                ypst = psum.tile([128, QC], f32, tag="acc", bufs=4,
                                 name="yps")
                yps_out[h2] = ypst
                for j in range(jmax + 1):
                    lhsT = Vaug[:, p, j, h2 * 65:(h2 + 1) * 65]
                    off = c * QC - j * KB
                    if off >= 0:
                        rhs = pt[p][(h2, j)][:, off:off + QC]
                        out = ypst[0:65, :]
                    else:
                        rhs = pt[p][(h2, j)][:, 0:QC + off]
                        out = ypst[0:65, -off:QC]
                    nc.tensor.matmul(out, lhsT, rhs,
                                     start=(j == 0), stop=(j == jmax))

            def emit_norm(p, c, h2, ypst):
                """normalize rows 0-63 by the ones-row (64) -> Yn.
                Broadcast of 1/den via a K=1 PE matmul so gpsimd's FIFO
                (which runs the causal masks) is never on this chain."""
                dsb = stream.tile([128, QC], f32, tag="dsb")
                nc.vector.tensor_copy(dsb[0:1, :], ypst[64:65, :])
                recip = stream.tile([128, QC], f32, tag="recip")
                nc.vector.reciprocal_approx_fast(
                    out=recip[0:1, :], in_=dsb[0:1, :])
                bc = stream.tile([128, QC], f32, tag="bc")
                nc.gpsimd.partition_broadcast(bc[0:64, :], recip[0:1, :])
                if h2 == 0:
                    nc.vector.tensor_tensor(
                        out=Yn[0:64, p, c * QC:(c + 1) * QC],
                        in0=ypst[0:64, :], in1=bc[0:64, :],
                        op=mybir.AluOpType.mult)
                else:
                    ytmp = stream.tile([128, QC], f16, tag="ytmp")
                    nc.vector.tensor_tensor(
                        out=ytmp[0:64, :], in0=ypst[0:64, :],
                        in1=bc[0:64, :], op=mybir.AluOpType.mult)
                    nc.sync.dma_start(
                        out=Yn[64:128, p, c * QC:(c + 1) * QC],
                        in_=ytmp[0:64, :])

            def emit_proj(tt2):
                """Projection for a 256-token block (4 MMs, 2 evacs)."""
                for tt in range(2 * tt2, 2 * tt2 + 2):
                    for half in range(2):
                        pout = psum.tile([128, 512], f32, tag="acc",
                                         bufs=4, name="pout")
                        for pp in range(2):
                            nc.tensor.matmul(
                                pout, Yn[:, pp, tt * 128:(tt + 1) * 128],
                                wp_sb[:, pp, half * 512:(half + 1) * 512],
                                start=(pp == 0), stop=(pp == 1))
                        yout = stream.tile([128, 512], f16, tag="yo")
                        nc.vector.tensor_copy(yout, pout)
                        nc.sync.dma_start(
                            out=y[tt * 128:(tt + 1) * 128,
                                  half * 512:(half + 1) * 512],
                            in_=yout)

            # ---------- phase 1 lead-in: pair-0 Q + first K ----------
            emit_qk(0, 0, 0)
            emit_qk(1, 0, 0)
            # deferred init ops (off the startup critical path)
            ident = const.tile([128, 128], f16)
            make_identity(nc, ident)
            ones_cols = Vaug.rearrange("p pp J (h x) -> p pp J h x",
                                       x=65)[:, :, :, :, 64]
            nc.gpsimd.memset(ones_cols, 1.0)
            emit_qk(2, 0, 0)
            emit_qk(3, 0, 0)
            emit_qk(0, 0, 1)

            # ---------- windowed attention schedule ----------
            # Score+exp windows (1024-wide) pace the scalar engine; one PE
            # filler granule (~1.5-2us) is emitted per window so the PE never
            # idles long enough to cold-throttle. PV/norm/proj granules are
            # scheduled a couple of windows after their last input exp.
            fillers = [
                lambda: emit_qk(1, 0, 1),        # K-p0 c1 (scores j>=4)
                lambda: emit_v(0, 0),            # PV-p0 c0
                lambda: emit_qk(2, 0, 1),        # K-p0 c2 (scores j>=8)
                lambda: emit_qk(0, 1, 0),
                lambda: emit_qk(3, 0, 1),        # K-p0 c3 (scores j>=12)
                lambda: emit_v(1, 0),            # PV-p0 c1
                lambda: emit_qk(0, 1, 1),
                lambda: emit_qk(1, 1, 0),
                lambda: emit_v(2, 0),            # PV-p0 c2
                lambda: emit_qk(1, 1, 1),
                lambda: emit_qk(2, 1, 0),
                lambda: emit_v(3, 0),            # PV-p0 c3
                lambda: emit_qk(2, 1, 1),
                lambda: emit_qk(3, 1, 0),
                lambda: emit_qk(3, 1, 1),
            ]
            fillers_p1 = [
                lambda: emit_v(0, 1),            # PV-p1 c0
                lambda: emit_v(1, 1),
                lambda: emit_v(2, 1),
                lambda: emit_v(3, 1),
            ]
            yps_box = [{}, {}]  # per pair: h2 -> yps tile

            def mk_pv(p, c, h2):
                def f():
                    emit_pv_head(p, c, h2, yps_box[p])
                    emit_norm(p, c, h2, yps_box[p][h2])
                return f

            def mk_proj(tt2):
                return lambda: emit_proj(tt2)

            # pair-1 score windows pre-emitted into pair-0's last slots;
            # window (j, 0) reuses pt buffers whose columns are consumed by
            # PV-p0 chunks <= 2, all done by then (subtile WAR tracking).
            PRE = {18: [0], 21: [1], 22: [2, 3], 23: [4]}
            pre_js = [j for js in PRE.values() for j in js]

            for p in range(NP):
                # per-pair window list with last-window index per chunk
                windows = []
                for j in range(NKB):
                    L = T - j * KB
                    for w0 in range(0, L, 1024):
                        if p == 1 and w0 == 0 and j in pre_js:
                            continue
                        windows.append((j, w0, L))
                ready_at = {}
                for wi, (j, w0, L) in enumerate(windows):
                    if j % 4 == 3 and w0 + 1024 >= L:
                        ready_at[j // 4] = wi
                # PV/norm h2=0 two windows after the chunk's last exp,
                # h2=1 one window later; projection (pair 1) after that.
                # Chunk 3 flushes right after the final window of the pair.
                sched = {}
                for c, wi in ready_at.items():
                    d = 1 if c < 3 else 0
                    sched.setdefault(wi + d, []).append(mk_pv(p, c, 0))
                    sched.setdefault(wi + d + (1 if c < 3 else 0),
                                     []).append(mk_pv(p, c, 1))
                    if p == 1:
                        sched.setdefault(wi + d + 1, []).append(
                            mk_proj(2 * c))
                        sched.setdefault(wi + d + 2, []).append(
                            mk_proj(2 * c + 1))

                nwin = len(windows)
                fq = fillers if p == 0 else fillers_p1
                for wi, (j, w0, L) in enumerate(windows):
                    emit_score_window(p, j, w0)
                    for f in sched.pop(wi, []):
                        f()
                    if fq and (p == 0 or wi % 2 == 0):
                        fq.pop(0)()
                    if p == 0:
                        for j1 in PRE.get(wi, []):
                            emit_score_window(1, j1, 0)
                # flush anything scheduled past the last window
                for wi in sorted(k for k in sched if k >= nwin):
                    for f in sched.pop(wi):
                        f()
                assert not sched, sched
                for f in fq:
                    f()
                fq.clear()

    nc.finalize()
    return nc


def _host_prep(x, cos, sin, w_attn, b_attn, w_proj):
    """Shared + per-core input arrays (all fp16)."""
    x = np.asarray(x, dtype=np.float32)
    xT16 = [np.ascontiguousarray(x[b].T).astype(np.float16) for b in range(B)]

    cos = np.asarray(cos, dtype=np.float32)
    sin = np.asarray(sin, dtype=np.float32)
    d = np.arange(128) % 64
    freq_i = d // 2
    sign = np.where(d % 2 == 0, -1.0, 1.0).astype(np.float32)
    cos_exp = cos[:, freq_i].T.astype(np.float16)            # [128, T]
    sin_exp = (sign[:, None] * sin[:, freq_i].T).astype(np.float16)

    pswap = np.zeros((128, 128), dtype=np.float16)
    idx = np.arange(128)
    pswap[idx ^ 1, idx] = 1.0

    # [k, q] = -100 where q < k (pre-exp additive causal mask)
    mneg = np.where(np.arange(128)[None, :] < np.arange(128)[:, None],
                    np.float16(-100.0), np.float16(0.0)).astype(np.float16)

    w_attn = np.asarray(w_attn, dtype=np.float32)
    w_proj = np.asarray(w_proj, dtype=np.float32)
    scale = 1.0 / np.sqrt(HD)

    per_core = []
    for m in range(N_CORES):
        hg = m % TPC
        heads = [4 * hg + k for k in range(4)]
        cols = []
        # groups: Q-p0, K-p0, Q-p1, K-p1, V-p0, V-p1 (heads pair-major)
        for p in range(NP):
            for g in range(2):  # Q, K
                for hh in heads[2 * p:2 * p + 2]:
                    blk = w_attn[:, g * C + hh * HD: g * C + (hh + 1) * HD]
                    if g == 0:
                        blk = blk * scale
                    cols.append(blk)
        for p in range(NP):
            for hh in heads[2 * p:2 * p + 2]:
                cols.append(w_attn[:, 2 * C + hh * HD: 2 * C + (hh + 1) * HD])
        w_stack = np.concatenate(cols, axis=1).astype(np.float16)
        wp_m = np.concatenate(
            [w_proj[hh * HD:(hh + 1) * HD, :] for hh in heads],
            axis=0).astype(np.float16)
        per_core.append((w_stack, wp_m))
    return xT16, cos_exp, sin_exp, pswap, mneg, per_core


def _build_in_maps(inputs):
    xT16, cos_exp, sin_exp, pswap, mneg, per_core = _host_prep(
        inputs["x"], inputs["cos"], inputs["sin"], inputs["w_attn"],
        inputs["b_attn"], inputs["w_proj"])
    in_maps = []
    for m in range(N_CORES):
        w_stack, wp_m = per_core[m]
        in_maps.append({
            "xT": xT16[m // TPC], "wqkv": w_stack, "wp": wp_m,
            "cos_d": cos_exp, "sin_d": sin_exp, "pswap": pswap,
            "maskneg": mneg,
        })
    return in_maps


def kernel(x, cos, sin, w_attn, b_attn, w_proj, b_proj):
    from concourse.bass_utils import run_bass_kernel_spmd

    b_attn = np.asarray(b_attn, dtype=np.float32)
    assert not np.any(b_attn), "nonzero b_attn not supported by this kernel"

    in_maps = _build_in_maps({
        "x": x, "cos": cos, "sin": sin, "w_attn": w_attn,
        "b_attn": b_attn, "w_proj": w_proj})

    if "nc" not in _CACHE:
        _CACHE["nc"] = _build_bass()
    nc = _CACHE["nc"]

    res = run_bass_kernel_spmd(nc, in_maps, core_ids=list(range(N_CORES)))
    _CACHE["last_result"] = res

    yf = np.zeros((B, T, C), dtype=np.float64)
    for m in range(N_CORES):
        yf[m // TPC] += res.results[m]["y"].astype(np.float64)
    yf = yf + np.asarray(b_proj, dtype=np.float64)[None, None, :]
    return yf.astype(np.float32)


# revision 35
# speedup vs baseline: 1.0208x; 1.0092x over previous
"""Causal self-attention with rotary embeddings on 8 Trainium2 NeuronCores.

v2: 2-way batch data-parallel x 4-way head tensor-parallel.
Core m handles batch m//4 and heads 4*(m%4)..4*(m%4)+3 (two pairs).
Each core computes qkv for its 4 heads, rotary, causal attention, and a
partial output projection (its 256 rows of w_proj) over its batch's
2048 tokens; the host sums 4 partials per batch.

Device-side layout (per core; heads within a pair at partitions 0-63 / 64-127):
  - Everything "transposed": Q^T/K^T stored [d(128), pair, t(2048)].
  - Scores S^T = K_blk @ Q^T -> [k(128), q]; the two heads of a pair issue
    back-to-back K=64 matmuls into different PE row groups (tile_position
    (0,0)/(64,0)) so they run concurrently.
  - exp on the scalar engine (PSUM fp32 -> SBUF fp16 pt tiles); softmax's
    k-sum folded into P@V via a ones-augmented V column (denominator row).
  - Rotary via pair-swap permutation matmul: rot(q) = cos*q + sin_sgn*(Pswap@q).
  - V transposed to t-major via hardware DMA transpose (XBAR), not the PE.
  - PV accumulates the causal triangle with narrow diagonal-block matmuls
    (has_written semantics preserve untouched columns); no zero-padding.
  - Emission interleaves pair-1 QK, V-groups and PV/projection chunks into
    the exp-paced score stream so the PE never head-of-line blocks.
"""

import numpy as np

B, T, C, H = 2, 2048, 1024, 16
HD = C // H            # 64
N_CORES = 8
DP = 2                 # batch shards
TPC = 4                # head-group shards
NP = 2                 # pairs per core (4 heads)
TC = 512               # t-chunk
NTC = T // TC          # 4
KB = 128               # k-block
NKB = T // KB          # 16
QC = 512               # q-chunk for PV
NQC = T // QC          # 4

_CACHE = {}


def _build_bass():
    import concourse.bacc as bacc
    import concourse.mybir as mybir
    import concourse.tile as tile
    from concourse.masks import make_identity

    f16 = mybir.dt.float16
    f32 = mybir.dt.float32

    nc = bacc.Bacc()

    # host pre-swizzles x/wqkv/wp into per-partition SBUF layout so every
    # input DMA reads contiguous DRAM (the naive layouts gather rows 512KB
    # apart and run at ~1/3 bandwidth, stalling the lead-in)
    xT = nc.dram_tensor("xT", [128, NTC * (C // 128) * TC], f16,
                        kind="ExternalInput")
    wqkv = nc.dram_tensor("wqkv", [128, 6 * (C // 128) * 128], f16,
                          kind="ExternalInput")
    wp = nc.dram_tensor("wp", [128, 2 * C], f16, kind="ExternalInput")
    cos_d = nc.dram_tensor("cos_d", [128, T], f16, kind="ExternalInput")
    sin_d = nc.dram_tensor("sin_d", [128, T], f16, kind="ExternalInput")
    pswap = nc.dram_tensor("pswap", [128, 128], f16, kind="ExternalInput")
    maskneg = nc.dram_tensor("maskneg", [128, 128], f16, kind="ExternalInput")
    y = nc.dram_tensor("y", [T, C], f16, kind="ExternalOutput")

    CCH = C // 128  # 8 contraction chunks

    with tile.TileContext(nc) as tc:
        with (
            tc.tile_pool(name="const", bufs=1) as const,
            tc.tile_pool(name="persist", bufs=1) as persist,
            tc.tile_pool(name="ptp", bufs=1) as ptp,
            tc.tile_pool(name="stream", bufs=2) as stream,
            tc.tile_pool(name="psum", bufs=1, space="PSUM") as psum,
        ):
            # ---- constants (ordered so the first QKV group starts ASAP) ----
            xT_r = xT.rearrange("p (i cc t) -> p i cc t", cc=CCH, t=TC)
            x_all = persist.tile([128, NTC, CCH, TC], f16)
            nc.sync.dma_start(out=x_all[:, 0], in_=xT_r[:, 0])
            wqkv_sb = const.tile([128, 6, CCH, 128], f16)
            wqkv_r = wqkv.rearrange("p (g cc j) -> p g cc j", cc=CCH, j=128)
            nc.sync.dma_start(out=wqkv_sb[:, 0], in_=wqkv_r[:, 0])
            nc.sync.dma_start(out=wqkv_sb[:, 1], in_=wqkv_r[:, 1])
            nc.sync.dma_start(out=x_all[:, 1], in_=xT_r[:, 1])
            pswap_sb = const.tile([128, 128], f16)
            nc.sync.dma_start(out=pswap_sb, in_=pswap[:, :])
            cos_sb = const.tile([128, T], f16)
            nc.sync.dma_start(out=cos_sb, in_=cos_d[:, :])
            sin_sb = const.tile([128, T], f16)
            nc.sync.dma_start(out=sin_sb, in_=sin_d[:, :])
            for i in range(2, NTC):
                nc.sync.dma_start(out=x_all[:, i], in_=xT_r[:, i])
            nc.sync.dma_start(out=wqkv_sb[:, 2:6], in_=wqkv_r[:, 2:6])
            wp_sb = const.tile([128, 2, C], f16)
            nc.sync.dma_start(out=wp_sb,
                              in_=wp.rearrange("p (pp c) -> p pp c", c=C))
            # maskneg[k, q] = -100 where q < k, else 0 (added to diag scores
            # pre-exp via an identity matmul, so masking costs PE only)
            mneg_sb = const.tile([128, 128], f16)
            nc.sync.dma_start(out=mneg_sb, in_=maskneg[:, :])

            # ---- persistent tensors ----
            QrotT = persist.tile([128, NP, T], f16)
            KrotT = persist.tile([128, NP, T], f16)
            # V t-major per (pair, k-block): [V_A(64) | ones | V_B(64) | ones]
            Vaug = persist.tile([128, NP, NKB, 130], f16)
            Yn = persist.tile([128, NP, T], f16)

            # ---------- emission helpers ----------
            def emit_qk(i, p, gk):
                """QKV group for (chunk i, pair p, Q:gk=0/K:gk=1) + rotary."""
                ts = slice(i * TC, (i + 1) * TC)
                g = 2 * p + gk
                dst = QrotT if gk == 0 else KrotT
                acc = psum.tile([128, TC], f32, tag="acc", bufs=4, name="acc")
                for cc in range(CCH):
                    nc.tensor.matmul(
                        acc, wqkv_sb[:, g, cc, :], x_all[:, i, cc, :],
                        start=(cc == 0), stop=(cc == CCH - 1))
                graw = stream.tile([128, TC], f16, tag="graw", bufs=3)
                nc.vector.tensor_copy(graw, acc)
                swp = psum.tile([128, TC], f32, tag="acc", bufs=4, name="swp")
                nc.tensor.matmul(swp, pswap_sb, graw, start=True, stop=True)
                t1 = stream.tile([128, TC], f16, tag="t1")
                nc.vector.tensor_mul(t1, graw, cos_sb[:, ts])
                t2 = stream.tile([128, TC], f16, tag="t2")
                nc.vector.tensor_mul(t2, swp, sin_sb[:, ts])
                nc.vector.tensor_add(dst[:, p, ts], t1, t2)

            def emit_v(i, p):
                """V group for (chunk i, pair p): matmul + DMA transpose."""
                g = 4 + p
                acc = psum.tile([128, TC], f32, tag="acc", bufs=4, name="vacc")
                for cc in range(CCH):
                    nc.tensor.matmul(
                        acc, wqkv_sb[:, g, cc, :], x_all[:, i, cc, :],
                        start=(cc == 0), stop=(cc == CCH - 1))
                vtmp = stream.tile([128, TC], f16, tag="vtmp")
                nc.vector.tensor_copy(vtmp, acc)
                vt4 = stream.tile([128, 4, 128], f16, tag="vt4")
                nc.sync.dma_start_transpose(out=vt4[:, :, :], in_=vtmp)
                # vt4[tlo, thi, d]; d 0-63 head A, 64-127 head B
                Vr = Vaug.rearrange("p pp J (h x) -> p pp J h x", x=65)
                nc.vector.tensor_copy(
                    Vr[:, p, 4 * i:4 * i + 4, 0, 0:64], vt4[:, :, 0:64])
                nc.vector.tensor_copy(
                    Vr[:, p, 4 * i:4 * i + 4, 1, 0:64], vt4[:, :, 64:128])

            # pt tiles per pair (the tag ring shares buffers across pairs;
            # keeping separate dicts preserves each pair's dependency chain)
            pt = [{}, {}]

            def emit_score_window(p, j, w0):
                """One 1024-wide score+exp window for both heads of pair p."""
                L = T - j * KB
                k0 = j * KB
                if w0 == 0:
                    ptA = ptp.tile([128, L], f16, tag=f"pt0_{j}", bufs=1,
                                   name="ptA")
                    ptB = ptp.tile([128, L], f16, tag=f"pt1_{j}", bufs=1,
                                   name="ptB")
                    pt[p][(0, j)] = ptA
                    pt[p][(1, j)] = ptB
                ptA, ptB = pt[p][(0, j)], pt[p][(1, j)]
                nw = min(1024, L - w0)
                stA = psum.tile([128, 1024], f32, tag="stA", bufs=1,
                                name="stA")
                stB = psum.tile([128, 1024], f32, tag="stB", bufs=1,
                                name="stB")
                for s0 in range(0, nw, 512):
                    ns = min(512, nw - s0)
                    q0 = k0 + w0 + s0
                    diag = (w0 == 0 and s0 == 0)
                    nc.tensor.matmul(
                        stA[:, s0:s0 + ns], KrotT[0:64, p, k0:k0 + 128],
                        QrotT[0:64, p, q0:q0 + ns], start=True,
                        stop=not diag)
                    nc.tensor.matmul(
                        stB[:, s0:s0 + ns], KrotT[64:128, p, k0:k0 + 128],
                        QrotT[64:128, p, q0:q0 + ns], start=True,
                        stop=not diag)
                    if diag:
                        # add -100 above the causal diagonal before exp
                        nc.tensor.matmul(stA[:, 0:128], ident, mneg_sb,
                                         start=False, stop=True)
                        nc.tensor.matmul(stB[:, 0:128], ident, mneg_sb,
                                         start=False, stop=True)
                nc.scalar.activation(
                    ptA[:, w0:w0 + nw], stA[:, 0:nw],
                    mybir.ActivationFunctionType.Exp)
                nc.scalar.activation(
                    ptB[:, w0:w0 + nw], stB[:, 0:nw],
                    mybir.ActivationFunctionType.Exp)

            def emit_pv_head(p, c, h2, yps_out):
                """PV accumulation for one head of one q-chunk (<=16 MMs)."""
                jmax = 4 * c + 3
                ypst = psum.tile([128, QC], f32, tag="acc", bufs=4,
                                 name="yps")
                yps_out[h2] = ypst
                for j in range(jmax + 1):
                    lhsT = Vaug[:, p, j, h2 * 65:(h2 + 1) * 65]
                    off = c * QC - j * KB
                    if off >= 0:
                        rhs = pt[p][(h2, j)][:, off:off + QC]
                        out = ypst[0:65, :]
                    else:
                        rhs = pt[p][(h2, j)][:, 0:QC + off]
                        out = ypst[0:65, -off:QC]
                    nc.tensor.matmul(out, lhsT, rhs,
                                     start=(j == 0), stop=(j == jmax))

            def emit_norm(p, c, h2, ypst):
                """normalize rows 0-63 by the ones-row (64) -> Yn.
                Broadcast of 1/den via a K=1 PE matmul so gpsimd's FIFO
                (which runs the causal masks) is never on this chain."""
                dsb = stream.tile([128, QC], f32, tag="dsb")
                nc.vector.tensor_copy(dsb[0:1, :], ypst[64:65, :])
                recip = stream.tile([128, QC], f32, tag="recip")
                nc.vector.reciprocal_approx_fast(
                    out=recip[0:1, :], in_=dsb[0:1, :])
                bc = stream.tile([128, QC], f32, tag="bc")
                nc.gpsimd.partition_broadcast(bc[0:64, :], recip[0:1, :])
                if h2 == 0:
                    nc.vector.tensor_tensor(
                        out=Yn[0:64, p, c * QC:(c + 1) * QC],
                        in0=ypst[0:64, :], in1=bc[0:64, :],
                        op=mybir.AluOpType.mult)
                else:
                    ytmp = stream.tile([128, QC], f16, tag="ytmp")
                    nc.vector.tensor_tensor(
                        out=ytmp[0:64, :], in0=ypst[0:64, :],
                        in1=bc[0:64, :], op=mybir.AluOpType.mult)
                    nc.sync.dma_start(
                        out=Yn[64:128, p, c * QC:(c + 1) * QC],
                        in_=ytmp[0:64, :])

            def emit_proj(tt2):
                """Projection for a 256-token block (4 MMs, 2 evacs)."""
                for tt in range(2 * tt2, 2 * tt2 + 2):
                    for half in range(2):
                        pout = psum.tile([128, 512], f32, tag="acc",
                                         bufs=4, name="pout")
                        for pp in range(2):
                            nc.tensor.matmul(
                                pout, Yn[:, pp, tt * 128:(tt + 1) * 128],
                                wp_sb[:, pp, half * 512:(half + 1) * 512],
                                start=(pp == 0), stop=(pp == 1))
                        yout = stream.tile([128, 512], f16, tag="yo")
                        nc.vector.tensor_copy(yout, pout)
                        nc.sync.dma_start(
                            out=y[tt * 128:(tt + 1) * 128,
                                  half * 512:(half + 1) * 512],
                            in_=yout)

            # ---------- phase 1 lead-in: pair-0 Q + first K ----------
            emit_qk(0, 0, 0)
            emit_qk(1, 0, 0)
            # deferred init ops (off the startup critical path)
            ident = const.tile([128, 128], f16)
            make_identity(nc, ident)
            ones_cols = Vaug.rearrange("p pp J (h x) -> p pp J h x",
                                       x=65)[:, :, :, :, 64]
            nc.gpsimd.memset(ones_cols, 1.0)
            emit_qk(2, 0, 0)
            emit_qk(3, 0, 0)
            emit_qk(0, 0, 1)

            # ---------- windowed attention schedule ----------
            # Score+exp windows (1024-wide) pace the scalar engine; one PE
            # filler granule (~1.5-2us) is emitted per window so the PE never
            # idles long enough to cold-throttle. PV/norm/proj granules are
            # scheduled a couple of windows after their last input exp.
            fillers = [
                lambda: emit_qk(1, 0, 1),        # K-p0 c1 (scores j>=4)
                lambda: emit_v(0, 0),            # PV-p0 c0
                lambda: emit_qk(2, 0, 1),        # K-p0 c2 (scores j>=8)
                lambda: emit_qk(0, 1, 0),
                lambda: emit_qk(3, 0, 1),        # K-p0 c3 (scores j>=12)
                lambda: emit_v(1, 0),            # PV-p0 c1
                lambda: emit_qk(0, 1, 1),
                lambda: emit_qk(1, 1, 0),
                lambda: emit_v(2, 0),            # PV-p0 c2
                lambda: emit_qk(1, 1, 1),
                lambda: emit_qk(2, 1, 0),
                lambda: emit_v(3, 0),            # PV-p0 c3
                lambda: emit_qk(2, 1, 1),
                lambda: emit_qk(3, 1, 0),
                lambda: emit_qk(3, 1, 1),
            ]
            fillers_p1 = [
                lambda: emit_v(0, 1),            # PV-p1 c0
                lambda: emit_v(1, 1),
                lambda: emit_v(2, 1),
                lambda: emit_v(3, 1),
            ]
            yps_box = [{}, {}]  # per pair: h2 -> yps tile

            def mk_pv(p, c, h2):
                def f():
                    emit_pv_head(p, c, h2, yps_box[p])
                    emit_norm(p, c, h2, yps_box[p][h2])
                return f

            def mk_proj(tt2):
                return lambda: emit_proj(tt2)

            # pair-1 score windows pre-emitted into pair-0's last slots;
            # window (j, 0) reuses pt buffers whose columns are consumed by
            # PV-p0 chunks <= 2, all done by then (subtile WAR tracking).
            PRE = {18: [0], 21: [1], 22: [2, 3], 23: [4]}
            pre_js = [j for js in PRE.values() for j in js]

            for p in range(NP):
                # per-pair window list with last-window index per chunk
                windows = []
                for j in range(NKB):
                    L = T - j * KB
                    for w0 in range(0, L, 1024):
                        if p == 1 and w0 == 0 and j in pre_js:
                            continue
                        windows.append((j, w0, L))
                ready_at = {}
                for wi, (j, w0, L) in enumerate(windows):
                    if j % 4 == 3 and w0 + 1024 >= L:
                        ready_at[j // 4] = wi
                # PV/norm h2=0 two windows after the chunk's last exp,
                # h2=1 one window later; projection (pair 1) after that.
                # Chunk 3 flushes right after the final window of the pair.
                sched = {}
                for c, wi in ready_at.items():
                    d = 1 if c < 3 else 0
                    sched.setdefault(wi + d, []).append(mk_pv(p, c, 0))
                    sched.setdefault(wi + d + (1 if c < 3 else 0),
                                     []).append(mk_pv(p, c, 1))
                    if p == 1:
                        sched.setdefault(wi + d + 1, []).append(
                            mk_proj(2 * c))
                        sched.setdefault(wi + d + 2, []).append(
                            mk_proj(2 * c + 1))

                nwin = len(windows)
                fq = fillers if p == 0 else fillers_p1
                for wi, (j, w0, L) in enumerate(windows):
                    emit_score_window(p, j, w0)
                    for f in sched.pop(wi, []):
                        f()
                    if fq and (p == 0 or wi % 2 == 0):
                        fq.pop(0)()
                    if p == 0:
                        for j1 in PRE.get(wi, []):
                            emit_score_window(1, j1, 0)
                # flush anything scheduled past the last window
                for wi in sorted(k for k in sched if k >= nwin):
                    for f in sched.pop(wi):
                        f()
                assert not sched, sched
                for f in fq:
                    f()
                fq.clear()

    nc.finalize()
    return nc


def _host_prep(x, cos, sin, w_attn, b_attn, w_proj):
    """Shared + per-core input arrays (all fp16)."""
    x = np.asarray(x, dtype=np.float32)
    # [p, i, cc, t] pre-swizzle: contiguous per-partition DMA rows
    xT16 = [np.ascontiguousarray(
        x[b].T.astype(np.float16).reshape(8, 128, NTC, TC)
        .transpose(1, 2, 0, 3).reshape(128, NTC * 8 * TC)) for b in range(B)]

    cos = np.asarray(cos, dtype=np.float32)
    sin = np.asarray(sin, dtype=np.float32)
    d = np.arange(128) % 64
    freq_i = d // 2
    sign = np.where(d % 2 == 0, -1.0, 1.0).astype(np.float32)
    cos_exp = cos[:, freq_i].T.astype(np.float16)            # [128, T]
    sin_exp = (sign[:, None] * sin[:, freq_i].T).astype(np.float16)

    pswap = np.zeros((128, 128), dtype=np.float16)
    idx = np.arange(128)
    pswap[idx ^ 1, idx] = 1.0

    # [k, q] = -100 where q < k (pre-exp additive causal mask)
    mneg = np.where(np.arange(128)[None, :] < np.arange(128)[:, None],
                    np.float16(-100.0), np.float16(0.0)).astype(np.float16)

    w_attn = np.asarray(w_attn, dtype=np.float32)
    w_proj = np.asarray(w_proj, dtype=np.float32)
    scale = 1.0 / np.sqrt(HD)

    per_core = []
    for m in range(N_CORES):
        hg = m % TPC
        heads = [4 * hg + k for k in range(4)]
        cols = []
        # groups: Q-p0, K-p0, Q-p1, K-p1, V-p0, V-p1 (heads pair-major)
        for p in range(NP):
            for g in range(2):  # Q, K
                for hh in heads[2 * p:2 * p + 2]:
                    blk = w_attn[:, g * C + hh * HD: g * C + (hh + 1) * HD]
                    if g == 0:
                        blk = blk * scale
                    cols.append(blk)
        for p in range(NP):
            for hh in heads[2 * p:2 * p + 2]:
                cols.append(w_attn[:, 2 * C + hh * HD: 2 * C + (hh + 1) * HD])
        w_stack = np.concatenate(cols, axis=1).astype(np.float16)
        w_stack = np.ascontiguousarray(
            w_stack.reshape(8, 128, 6, 128).transpose(1, 2, 0, 3)
            .reshape(128, 6 * 8 * 128))
        wp_m = np.concatenate(
            [w_proj[hh * HD:(hh + 1) * HD, :] for hh in heads],
            axis=0).astype(np.float16)
        wp_m = np.ascontiguousarray(
            wp_m.reshape(2, 128, C).transpose(1, 0, 2).reshape(128, 2 * C))
        per_core.append((w_stack, wp_m))
    return xT16, cos_exp, sin_exp, pswap, mneg, per_core


def _build_in_maps(inputs):
    xT16, cos_exp, sin_exp, pswap, mneg, per_core = _host_prep(
        inputs["x"], inputs["cos"], inputs["sin"], inputs["w_attn"],
        inputs["b_attn"], inputs["w_proj"])
    in_maps = []
    for m in range(N_CORES):
        w_stack, wp_m = per_core[m]
        in_maps.append({
            "xT": xT16[m // TPC], "wqkv": w_stack, "wp": wp_m,
            "cos_d": cos_exp, "sin_d": sin_exp, "pswap": pswap,
            "maskneg": mneg,
        })
    return in_maps


def kernel(x, cos, sin, w_attn, b_attn, w_proj, b_proj):
    from concourse.bass_utils import run_bass_kernel_spmd

    b_attn = np.asarray(b_attn, dtype=np.float32)
    assert not np.any(b_attn), "nonzero b_attn not supported by this kernel"

    in_maps = _build_in_maps({
        "x": x, "cos": cos, "sin": sin, "w_attn": w_attn,
        "b_attn": b_attn, "w_proj": w_proj})

    if "nc" not in _CACHE:
        _CACHE["nc"] = _build_bass()
    nc = _CACHE["nc"]

    res = run_bass_kernel_spmd(nc, in_maps, core_ids=list(range(N_CORES)))
    _CACHE["last_result"] = res

    yf = np.zeros((B, T, C), dtype=np.float64)
    for m in range(N_CORES):
        yf[m // TPC] += res.results[m]["y"].astype(np.float64)
    yf = yf + np.asarray(b_proj, dtype=np.float64)[None, None, :]
    return yf.astype(np.float32)


# revision 37
# speedup vs baseline: 1.0236x; 1.0028x over previous
"""Causal self-attention with rotary embeddings on 8 Trainium2 NeuronCores.

v2: 2-way batch data-parallel x 4-way head tensor-parallel.
Core m handles batch m//4 and heads 4*(m%4)..4*(m%4)+3 (two pairs).
Each core computes qkv for its 4 heads, rotary, causal attention, and a
partial output projection (its 256 rows of w_proj) over its batch's
2048 tokens; the host sums 4 partials per batch.

Device-side layout (per core; heads within a pair at partitions 0-63 / 64-127):
  - Everything "transposed": Q^T/K^T stored [d(128), pair, t(2048)].
  - Scores S^T = K_blk @ Q^T -> [k(128), q]; the two heads of a pair issue
    back-to-back K=64 matmuls into different PE row groups (tile_position
    (0,0)/(64,0)) so they run concurrently.
  - exp on the scalar engine (PSUM fp32 -> SBUF fp16 pt tiles); softmax's
    k-sum folded into P@V via a ones-augmented V column (denominator row).
  - Rotary via pair-swap permutation matmul: rot(q) = cos*q + sin_sgn*(Pswap@q).
  - V transposed to t-major via hardware DMA transpose (XBAR), not the PE.
  - PV accumulates the causal triangle with narrow diagonal-block matmuls
    (has_written semantics preserve untouched columns); no zero-padding.
  - Emission interleaves pair-1 QK, V-groups and PV/projection chunks into
    the exp-paced score stream so the PE never head-of-line blocks.
"""

import numpy as np

B, T, C, H = 2, 2048, 1024, 16
HD = C // H            # 64
N_CORES = 8
DP = 2                 # batch shards
TPC = 4                # head-group shards
NP = 2                 # pairs per core (4 heads)
TC = 512               # t-chunk
NTC = T // TC          # 4
KB = 128               # k-block
NKB = T // KB          # 16
QC = 512               # q-chunk for PV
NQC = T // QC          # 4

_CACHE = {}


def _build_bass():
    import concourse.bacc as bacc
    import concourse.mybir as mybir
    import concourse.tile as tile
    from concourse.masks import make_identity

    f16 = mybir.dt.float16
    f32 = mybir.dt.float32

    nc = bacc.Bacc()

    # host pre-swizzles x/wqkv/wp into per-partition SBUF layout so every
    # input DMA reads contiguous DRAM (the naive layouts gather rows 512KB
    # apart and run at ~1/3 bandwidth, stalling the lead-in)
    xT = nc.dram_tensor("xT", [128, NTC * (C // 128) * TC], f16,
                        kind="ExternalInput")
    wqkv = nc.dram_tensor("wqkv", [128, 6 * (C // 128) * 128], f16,
                          kind="ExternalInput")
    wp = nc.dram_tensor("wp", [128, 2 * C], f16, kind="ExternalInput")
    cos_d = nc.dram_tensor("cos_d", [128, T], f16, kind="ExternalInput")
    sin_d = nc.dram_tensor("sin_d", [128, T], f16, kind="ExternalInput")
    pswap = nc.dram_tensor("pswap", [128, 128], f16, kind="ExternalInput")
    maskneg = nc.dram_tensor("maskneg", [128, 128], f16, kind="ExternalInput")
    y = nc.dram_tensor("y", [T, C], f16, kind="ExternalOutput")

    CCH = C // 128  # 8 contraction chunks

    with tile.TileContext(nc) as tc:
        with (
            tc.tile_pool(name="const", bufs=1) as const,
            tc.tile_pool(name="persist", bufs=1) as persist,
            tc.tile_pool(name="ptp", bufs=1) as ptp,
            tc.tile_pool(name="stream", bufs=2) as stream,
            tc.tile_pool(name="psum", bufs=1, space="PSUM") as psum,
        ):
            # ---- constants (ordered so the first QKV group starts ASAP) ----
            xT_r = xT.rearrange("p (i cc t) -> p i cc t", cc=CCH, t=TC)
            x_all = persist.tile([128, NTC, CCH, TC], f16)
            nc.sync.dma_start(out=x_all[:, 0], in_=xT_r[:, 0])
            wqkv_sb = const.tile([128, 6, CCH, 128], f16)
            wqkv_r = wqkv.rearrange("p (g cc j) -> p g cc j", cc=CCH, j=128)
            nc.sync.dma_start(out=wqkv_sb[:, 0], in_=wqkv_r[:, 0])
            nc.sync.dma_start(out=wqkv_sb[:, 1], in_=wqkv_r[:, 1])
            nc.sync.dma_start(out=x_all[:, 1], in_=xT_r[:, 1])
            pswap_sb = const.tile([128, 128], f16)
            nc.sync.dma_start(out=pswap_sb, in_=pswap[:, :])
            cos_sb = const.tile([128, T], f16)
            nc.sync.dma_start(out=cos_sb, in_=cos_d[:, :])
            sin_sb = const.tile([128, T], f16)
            nc.sync.dma_start(out=sin_sb, in_=sin_d[:, :])
            for i in range(2, NTC):
                nc.sync.dma_start(out=x_all[:, i], in_=xT_r[:, i])
            nc.sync.dma_start(out=wqkv_sb[:, 2:6], in_=wqkv_r[:, 2:6])
            wp_sb = const.tile([128, 2, C], f16)
            nc.sync.dma_start(out=wp_sb,
                              in_=wp.rearrange("p (pp c) -> p pp c", c=C))
            # maskneg[k, q] = -100 where q < k, else 0 (added to diag scores
            # pre-exp via an identity matmul, so masking costs PE only)
            mneg_sb = const.tile([128, 128], f16)
            nc.sync.dma_start(out=mneg_sb, in_=maskneg[:, :])

            # ---- persistent tensors ----
            QrotT = persist.tile([128, NP, T], f16)
            KrotT = persist.tile([128, NP, T], f16)
            # V t-major per (pair, k-block): [V_A(64) | ones | V_B(64) | ones]
            Vaug = persist.tile([128, NP, NKB, 130], f16)
            Yn = persist.tile([128, NP, T], f16)

            # ---------- emission helpers ----------
            def emit_qk(i, p, gk):
                """QKV group for (chunk i, pair p, Q:gk=0/K:gk=1) + rotary."""
                ts = slice(i * TC, (i + 1) * TC)
                g = 2 * p + gk
                dst = QrotT if gk == 0 else KrotT
                acc = psum.tile([128, TC], f32, tag="acc", bufs=4, name="acc")
                for cc in range(CCH):
                    nc.tensor.matmul(
                        acc, wqkv_sb[:, g, cc, :], x_all[:, i, cc, :],
                        start=(cc == 0), stop=(cc == CCH - 1))
                graw = stream.tile([128, TC], f16, tag="graw", bufs=3)
                nc.vector.tensor_copy(graw, acc)
                swp = psum.tile([128, TC], f32, tag="acc", bufs=4, name="swp")
                nc.tensor.matmul(swp, pswap_sb, graw, start=True, stop=True)
                t1 = stream.tile([128, TC], f16, tag="t1")
                nc.vector.tensor_mul(t1, graw, cos_sb[:, ts])
                t2 = stream.tile([128, TC], f16, tag="t2")
                nc.vector.tensor_mul(t2, swp, sin_sb[:, ts])
                nc.vector.tensor_add(dst[:, p, ts], t1, t2)

            def emit_v(i, p):
                """V group for (chunk i, pair p): matmul + DMA transpose."""
                g = 4 + p
                acc = psum.tile([128, TC], f32, tag="acc", bufs=4, name="vacc")
                for cc in range(CCH):
                    nc.tensor.matmul(
                        acc, wqkv_sb[:, g, cc, :], x_all[:, i, cc, :],
                        start=(cc == 0), stop=(cc == CCH - 1))
                vtmp = stream.tile([128, TC], f16, tag="vtmp")
                nc.vector.tensor_copy(vtmp, acc)
                vt4 = stream.tile([128, 4, 128], f16, tag="vt4")
                nc.sync.dma_start_transpose(out=vt4[:, :, :], in_=vtmp)
                # vt4[tlo, thi, d]; d 0-63 head A, 64-127 head B
                Vr = Vaug.rearrange("p pp J (h x) -> p pp J h x", x=65)
                nc.vector.tensor_copy(
                    Vr[:, p, 4 * i:4 * i + 4, 0, 0:64], vt4[:, :, 0:64])
                nc.vector.tensor_copy(
                    Vr[:, p, 4 * i:4 * i + 4, 1, 0:64], vt4[:, :, 64:128])

            # pt tiles per pair (the tag ring shares buffers across pairs;
            # keeping separate dicts preserves each pair's dependency chain)
            pt = [{}, {}]

            def emit_score_window(p, j, w0):
                """One 1024-wide score+exp window for both heads of pair p."""
                L = T - j * KB
                k0 = j * KB
                if w0 == 0:
                    ptA = ptp.tile([128, L], f16, tag=f"pt0_{j}", bufs=1,
                                   name="ptA")
                    ptB = ptp.tile([128, L], f16, tag=f"pt1_{j}", bufs=1,
                                   name="ptB")
                    pt[p][(0, j)] = ptA
                    pt[p][(1, j)] = ptB
                ptA, ptB = pt[p][(0, j)], pt[p][(1, j)]
                nw = min(1024, L - w0)
                stA = psum.tile([128, 1024], f32, tag="stA", bufs=1,
                                name="stA")
                stB = psum.tile([128, 1024], f32, tag="stB", bufs=1,
                                name="stB")
                for s0 in range(0, nw, 512):
                    ns = min(512, nw - s0)
                    q0 = k0 + w0 + s0
                    diag = (w0 == 0 and s0 == 0)
                    nc.tensor.matmul(
                        stA[:, s0:s0 + ns], KrotT[0:64, p, k0:k0 + 128],
                        QrotT[0:64, p, q0:q0 + ns], start=True,
                        stop=not diag)
                    nc.tensor.matmul(
                        stB[:, s0:s0 + ns], KrotT[64:128, p, k0:k0 + 128],
                        QrotT[64:128, p, q0:q0 + ns], start=True,
                        stop=not diag)
                    if diag:
                        # add -100 above the causal diagonal before exp
                        nc.tensor.matmul(stA[:, 0:128], ident, mneg_sb,
                                         start=False, stop=True)
                        nc.tensor.matmul(stB[:, 0:128], ident, mneg_sb,
                                         start=False, stop=True)
                nc.scalar.activation(
                    ptA[:, w0:w0 + nw], stA[:, 0:nw],
                    mybir.ActivationFunctionType.Exp)
                nc.scalar.activation(
                    ptB[:, w0:w0 + nw], stB[:, 0:nw],
                    mybir.ActivationFunctionType.Exp)

            def emit_pv_head(p, c, h2, yps_out):
                """PV accumulation for one head of one q-chunk (<=16 MMs)."""
                jmax = 4 * c + 3
                ypst = psum.tile([128, QC], f32, tag="acc", bufs=4,
                                 name="yps")
                yps_out[h2] = ypst
                for j in range(jmax + 1):
                    lhsT = Vaug[:, p, j, h2 * 65:(h2 + 1) * 65]
                    off = c * QC - j * KB
                    if off >= 0:
                        rhs = pt[p][(h2, j)][:, off:off + QC]
                        out = ypst[0:65, :]
                    else:
                        rhs = pt[p][(h2, j)][:, 0:QC + off]
                        out = ypst[0:65, -off:QC]
                    nc.tensor.matmul(out, lhsT, rhs,
                                     start=(j == 0), stop=(j == jmax))

            def emit_norm(p, c, h2, ypst):
                """normalize rows 0-63 by the ones-row (64) -> Yn.
                Broadcast of 1/den via a K=1 PE matmul so gpsimd's FIFO
                (which runs the causal masks) is never on this chain."""
                dsb = stream.tile([128, QC], f32, tag="dsb")
                nc.vector.tensor_copy(dsb[0:1, :], ypst[64:65, :])
                recip = stream.tile([128, QC], f32, tag="recip")
                nc.vector.reciprocal_approx_fast(
                    out=recip[0:1, :], in_=dsb[0:1, :])
                bc = stream.tile([128, QC], f32, tag="bc")
                nc.gpsimd.partition_broadcast(bc[0:64, :], recip[0:1, :])
                if h2 == 0:
                    nc.vector.tensor_tensor(
                        out=Yn[0:64, p, c * QC:(c + 1) * QC],
                        in0=ypst[0:64, :], in1=bc[0:64, :],
                        op=mybir.AluOpType.mult)
                else:
                    ytmp = stream.tile([128, QC], f16, tag="ytmp")
                    nc.vector.tensor_tensor(
                        out=ytmp[0:64, :], in0=ypst[0:64, :],
                        in1=bc[0:64, :], op=mybir.AluOpType.mult)
                    nc.sync.dma_start(
                        out=Yn[64:128, p, c * QC:(c + 1) * QC],
                        in_=ytmp[0:64, :])

            def emit_proj(tt2):
                """Projection for a 256-token block (4 MMs, 2 evacs)."""
                for tt in range(2 * tt2, 2 * tt2 + 2):
                    for half in range(2):
                        pout = psum.tile([128, 512], f32, tag="acc",
                                         bufs=4, name="pout")
                        for pp in range(2):
                            nc.tensor.matmul(
                                pout, Yn[:, pp, tt * 128:(tt + 1) * 128],
                                wp_sb[:, pp, half * 512:(half + 1) * 512],
                                start=(pp == 0), stop=(pp == 1))
                        yout = stream.tile([128, 512], f16, tag="yo")
                        nc.vector.tensor_copy(yout, pout)
                        nc.sync.dma_start(
                            out=y[tt * 128:(tt + 1) * 128,
                                  half * 512:(half + 1) * 512],
                            in_=yout)

            # ---------- phase 1 lead-in: pair-0 Q + first K ----------
            emit_qk(0, 0, 0)
            emit_qk(1, 0, 0)
            # deferred init ops (off the startup critical path)
            ident = const.tile([128, 128], f16)
            make_identity(nc, ident)
            ones_cols = Vaug.rearrange("p pp J (h x) -> p pp J h x",
                                       x=65)[:, :, :, :, 64]
            nc.gpsimd.memset(ones_cols, 1.0)
            emit_qk(2, 0, 0)
            emit_qk(3, 0, 0)
            emit_qk(0, 0, 1)

            # ---------- windowed attention schedule ----------
            # Score+exp windows (1024-wide) pace the scalar engine; one PE
            # filler granule (~1.5-2us) is emitted per window so the PE never
            # idles long enough to cold-throttle. PV/norm/proj granules are
            # scheduled a couple of windows after their last input exp.
            fillers = [
                lambda: emit_qk(1, 0, 1),        # K-p0 c1 (scores j>=4)
                lambda: emit_v(0, 0),            # PV-p0 c0
                lambda: emit_qk(2, 0, 1),        # K-p0 c2 (scores j>=8)
                lambda: emit_qk(0, 1, 0),
                lambda: emit_qk(3, 0, 1),        # K-p0 c3 (scores j>=12)
                lambda: emit_v(1, 0),            # PV-p0 c1
                lambda: emit_qk(0, 1, 1),
                lambda: emit_qk(1, 1, 0),
                lambda: emit_v(2, 0),            # PV-p0 c2
                lambda: emit_qk(1, 1, 1),
                lambda: emit_qk(2, 1, 0),
                lambda: emit_v(3, 0),            # PV-p0 c3
                lambda: emit_qk(2, 1, 1),
                lambda: emit_qk(3, 1, 0),
                lambda: emit_qk(3, 1, 1),
            ]
            fillers_p1 = [
                lambda: emit_v(0, 1),            # PV-p1 c0
                lambda: emit_v(1, 1),
                lambda: emit_v(2, 1),
                lambda: emit_v(3, 1),
            ]
            yps_box = [{}, {}]  # per pair: h2 -> yps tile

            def mk_pv(p, c, h2):
                def f():
                    emit_pv_head(p, c, h2, yps_box[p])
                    emit_norm(p, c, h2, yps_box[p][h2])
                return f

            def mk_proj(tt2):
                return lambda: emit_proj(tt2)

            # pair-1 score windows pre-emitted into pair-0's last slots;
            # window (j, 0) reuses pt buffers whose columns are consumed by
            # PV-p0 chunks <= 2, all done by then (subtile WAR tracking).
            PRE = {17: [0], 20: [1, 2], 21: [3], 22: [4]}
            pre_js = [j for js in PRE.values() for j in js]

            for p in range(NP):
                # per-pair window list with last-window index per chunk
                windows = []
                for j in range(NKB):
                    L = T - j * KB
                    for w0 in range(0, L, 1024):
                        if p == 1 and w0 == 0 and j in pre_js:
                            continue
                        windows.append((j, w0, L))
                ready_at = {}
                for wi, (j, w0, L) in enumerate(windows):
                    if j % 4 == 3 and w0 + 1024 >= L:
                        ready_at[j // 4] = wi
                # PV/norm h2=0 two windows after the chunk's last exp,
                # h2=1 one window later; projection (pair 1) after that.
                # Chunk 3 flushes right after the final window of the pair.
                sched = {}
                for c, wi in ready_at.items():
                    d = 1 if c < 2 else 0
                    sched.setdefault(wi + d, []).append(mk_pv(p, c, 0))
                    sched.setdefault(wi + d + (1 if c < 2 else 0),
                                     []).append(mk_pv(p, c, 1))
                    if p == 1:
                        sched.setdefault(wi + d + 1, []).append(
                            mk_proj(2 * c))
                        sched.setdefault(wi + d + 2, []).append(
                            mk_proj(2 * c + 1))

                nwin = len(windows)
                fq = fillers if p == 0 else fillers_p1
                for wi, (j, w0, L) in enumerate(windows):
                    emit_score_window(p, j, w0)
                    for f in sched.pop(wi, []):
                        f()
                    if fq and (p == 0 or wi % 2 == 0):
                        fq.pop(0)()
                    if p == 0:
                        for j1 in PRE.get(wi, []):
                            emit_score_window(1, j1, 0)
                # flush anything scheduled past the last window
                for wi in sorted(k for k in sched if k >= nwin):
                    for f in sched.pop(wi):
                        f()
                assert not sched, sched
                for f in fq:
                    f()
                fq.clear()

    nc.finalize()
    return nc


def _host_prep(x, cos, sin, w_attn, b_attn, w_proj):
    """Shared + per-core input arrays (all fp16)."""
    x = np.asarray(x, dtype=np.float32)
    # [p, i, cc, t] pre-swizzle: contiguous per-partition DMA rows
    xT16 = [np.ascontiguousarray(
        x[b].T.astype(np.float16).reshape(8, 128, NTC, TC)
        .transpose(1, 2, 0, 3).reshape(128, NTC * 8 * TC)) for b in range(B)]

    cos = np.asarray(cos, dtype=np.float32)
    sin = np.asarray(sin, dtype=np.float32)
    d = np.arange(128) % 64
    freq_i = d // 2
    sign = np.where(d % 2 == 0, -1.0, 1.0).astype(np.float32)
    cos_exp = cos[:, freq_i].T.astype(np.float16)            # [128, T]
    sin_exp = (sign[:, None] * sin[:, freq_i].T).astype(np.float16)

    pswap = np.zeros((128, 128), dtype=np.float16)
    idx = np.arange(128)
    pswap[idx ^ 1, idx] = 1.0

    # [k, q] = -100 where q < k (pre-exp additive causal mask)
    mneg = np.where(np.arange(128)[None, :] < np.arange(128)[:, None],
                    np.float16(-100.0), np.float16(0.0)).astype(np.float16)

    w_attn = np.asarray(w_attn, dtype=np.float32)
    w_proj = np.asarray(w_proj, dtype=np.float32)
    scale = 1.0 / np.sqrt(HD)

    per_core = []
    for m in range(N_CORES):
        hg = m % TPC
        heads = [4 * hg + k for k in range(4)]
        cols = []
        # groups: Q-p0, K-p0, Q-p1, K-p1, V-p0, V-p1 (heads pair-major)
        for p in range(NP):
            for g in range(2):  # Q, K
                for hh in heads[2 * p:2 * p + 2]:
                    blk = w_attn[:, g * C + hh * HD: g * C + (hh + 1) * HD]
                    if g == 0:
                        blk = blk * scale
                    cols.append(blk)
        for p in range(NP):
            for hh in heads[2 * p:2 * p + 2]:
                cols.append(w_attn[:, 2 * C + hh * HD: 2 * C + (hh + 1) * HD])
        w_stack = np.concatenate(cols, axis=1).astype(np.float16)
        w_stack = np.ascontiguousarray(
            w_stack.reshape(8, 128, 6, 128).transpose(1, 2, 0, 3)
            .reshape(128, 6 * 8 * 128))
        wp_m = np.concatenate(
            [w_proj[hh * HD:(hh + 1) * HD, :] for hh in heads],
            axis=0).astype(np.float16)
        wp_m = np.ascontiguousarray(
            wp_m.reshape(2, 128, C).transpose(1, 0, 2).reshape(128, 2 * C))
        per_core.append((w_stack, wp_m))
    return xT16, cos_exp, sin_exp, pswap, mneg, per_core


def _build_in_maps(inputs):
    xT16, cos_exp, sin_exp, pswap, mneg, per_core = _host_prep(
        inputs["x"], inputs["cos"], inputs["sin"], inputs["w_attn"],
        inputs["b_attn"], inputs["w_proj"])
    in_maps = []
    for m in range(N_CORES):
        w_stack, wp_m = per_core[m]
        in_maps.append({
            "xT": xT16[m // TPC], "wqkv": w_stack, "wp": wp_m,
            "cos_d": cos_exp, "sin_d": sin_exp, "pswap": pswap,
            "maskneg": mneg,
        })
    return in_maps


def kernel(x, cos, sin, w_attn, b_attn, w_proj, b_proj):
    from concourse.bass_utils import run_bass_kernel_spmd

    b_attn = np.asarray(b_attn, dtype=np.float32)
    assert not np.any(b_attn), "nonzero b_attn not supported by this kernel"

    in_maps = _build_in_maps({
        "x": x, "cos": cos, "sin": sin, "w_attn": w_attn,
        "b_attn": b_attn, "w_proj": w_proj})

    if "nc" not in _CACHE:
        _CACHE["nc"] = _build_bass()
    nc = _CACHE["nc"]

    res = run_bass_kernel_spmd(nc, in_maps, core_ids=list(range(N_CORES)))
    _CACHE["last_result"] = res

    yf = np.zeros((B, T, C), dtype=np.float64)
    for m in range(N_CORES):
        yf[m // TPC] += res.results[m]["y"].astype(np.float64)
    yf = yf + np.asarray(b_proj, dtype=np.float64)[None, None, :]
    return yf.astype(np.float32)
